# revision 9
# baseline (speedup 1.0000x reference)
"""EMA VectorQuantizer forward pass on 8 TRN2 NeuronCores (Bass/Tile).

Data-parallel over tokens: z [4,64,16,32,32] -> 65536 tokens of dim 64,
8192 tokens per core (channel-major shard [64, 8192] is a natural slice
of z's layout). The [4096,64] codebook is replicated. Per core:
  scores = 2*z@e.T via PE fp32 matmuls (tokens on partitions, codes on
  free dim), top-8 via DVE max8/max_index, then an exact fp32 re-ranking
  of the 8 candidates replicating jax's rounding of
  d = (||z||^2 + ||e||^2) - 2*z@e.T (the (zz+ee) double-rounding decides
  ~3% of tokens' grid-ties, so it must be emulated bit-exactly).
  z_q by indirect row gather; counts/embed_sum by per-tile dedup
  (selection-matrix matmul) + serialized indirect scatter-add DMAs; the
  per-core partial stats are AllReduce'd and the EMA buffer update is
  computed redundantly on every core.
"""

import numpy as np

import bass_rust
import concourse.bass as bass
import concourse.mybir as mybir
import concourse.tile as tile_mod
from concourse.bass import IndirectOffsetOnAxis
from concourse.masks import make_identity
from concourse.tile import TileContext
from concourse.tile_rust import add_dep_helper
from concourse.bass_utils import run_bass_kernel_spmd

N_CORES = 8
N_E = 4096
E_DIM = 64
T_LOC = 8192          # tokens per core
N_TILES = T_LOC // 128
BETA = 0.25
DECAY = 0.99
EPS = 1e-05
N_TOK = 65536
STATS_ROWS = 4224     # 4096 codes + spare rows for scalar partials
F32 = mybir.dt.float32

# ---------------------------------------------------------------------------
# workaround 1: walrus in this container rejects >1 sem wait on the
# TileContext tail drain — pre-absorb the global-clock waits one per drain.

def _patched_drain_and_barrier(self, tick_clock, wait_clock):
    nc = self.nc
    vc = tick_clock.global_clock
    nonzero = [(i, vc[i]) for i in range(len(vc)) if vc[i] > 0]
    for i, t in nonzero:
        pvc = bass_rust.VectorClock([0] * len(vc))
        pvc.require_at_least(i, t)
        nop = nc.sync.drain()
        wait_clock.add_sem_waits(nop.ins, bass_rust.ScopedClock({None: pvc}))
    nc.sync.drain()
    nc.all_engine_barrier()
    assert self.sems is not None
    popped = nc._tile_sem_poison_stack.pop()
    assert popped is self._sem_poison
    nc.clear_and_free_semaphores(list(self.sems.allocated().values()))
    nc.all_engine_barrier()


tile_mod.TileContext._drain_and_barrier = _patched_drain_and_barrier

# ---------------------------------------------------------------------------
# workaround 2: same walrus cap on every other instruction — hoist excess
# semaphore waits onto same-engine NoOps inserted right before it.

_wsplit_ctr = [0]


def _split_excess_waits(nc, max_sem_waits=1):
    for f in nc.m.functions:
        for bb in f.blocks:
            insts = bb.instructions
            new = []
            changed = False
            for inst in insts:
                si = inst.sync_info
                waits = list(si.on_wait) if (si and si.on_wait) else []
                sem_w = [w for w in waits if w.sync_type == "semaphore"]
                other_w = [w for w in waits if w.sync_type != "semaphore"]
                keep = max(0, max_sem_waits - len(other_w))
                if len(sem_w) > keep:
                    excess = sem_w[: len(sem_w) - keep]
                    kept = sem_w[len(sem_w) - keep:]
                    for w in excess:
                        _wsplit_ctr[0] += 1
                        nop = mybir.InstNoOp(
                            name=f"I-wsplit-{_wsplit_ctr[0]}", ins=[], outs=[]
                        )
                        nop.engine = inst.engine
                        nop.sync_info = mybir.SyncInfo(on_wait=[w], on_update=[])
                        new.append(nop)
                    inst.sync_info = mybir.SyncInfo(
                        on_wait=other_w + kept,
                        on_update=list(si.on_update) if si.on_update else [],
                    )
                    changed = True
                new.append(inst)
            if changed:
                insts[:] = new


# ---------------------------------------------------------------------------

def build_nc():
    nc = bass.Bass(trn_type="TRN2", num_devices=N_CORES)
    alu = mybir.AluOpType

    zT_in = nc.dram_tensor("zT", [E_DIM, T_LOC], F32, kind="ExternalInput")
    w_in = nc.dram_tensor("embed_w", [N_E, E_DIM], F32, kind="ExternalInput")
    cs_in = nc.dram_tensor("cluster_size", [N_E], F32, kind="ExternalInput")
    ea_in = nc.dram_tensor("embed_avg", [N_E, E_DIM], F32, kind="ExternalInput")

    out_sh = nc.dram_tensor("out_sh", [E_DIM, T_LOC], F32, kind="ExternalOutput")
    idx_out = nc.dram_tensor("idx_t", [128, N_TILES], mybir.dt.int32,
                             kind="ExternalOutput")
    loss_out = nc.dram_tensor("loss", [1, 1], F32, kind="ExternalOutput")
    ppl_out = nc.dram_tensor("ppl", [1, 1], F32, kind="ExternalOutput")
    nemb_out = nc.dram_tensor("new_embed", [N_E, E_DIM], F32, kind="ExternalOutput")
    dbg_staged = nc.dram_tensor("dbg_staged", [128, N_E], F32, kind="ExternalOutput")
    dbg_e2T = nc.dram_tensor("dbg_e2T", [E_DIM, N_E], F32, kind="ExternalOutput")
    ncs_out = nc.dram_tensor("new_cs", [N_E], F32, kind="ExternalOutput")
    nea_out = nc.dram_tensor("new_ea", [N_E, E_DIM], F32, kind="ExternalOutput")

    with TileContext(nc) as tc:
        with (
            tc.tile_pool(name="big", bufs=1) as bigp,      # persistent SBUF
            tc.tile_pool(name="wrk", bufs=2) as wrk,       # per-tile rotating
            tc.tile_pool(name="stg", bufs=2) as stg,       # staged scores
            tc.tile_pool(name="ps_sc", bufs=2, space="PSUM") as ps_sc,
            tc.tile_pool(name="ps_tp", bufs=1, space="PSUM") as ps_tp,
            tc.tile_pool(name="dram", bufs=1, space="DRAM") as dram,
        ):
            # ---------------- constants / setup ----------------
            ident = bigp.tile([128, 128], F32)
            make_identity(nc, ident[:, :])
            iota_row_i = bigp.tile([128, 128], mybir.dt.int32)
            nc.gpsimd.iota(iota_row_i[:, :], pattern=[[1, 128]], base=0,
                           channel_multiplier=0)
            iota_row = bigp.tile([128, 128], F32)
            nc.vector.tensor_copy(iota_row[:, :], iota_row_i[:, :])
            iota_p_i = bigp.tile([128, 1], mybir.dt.int32)
            nc.gpsimd.iota(iota_p_i[:, :], pattern=[[1, 1]], base=0,
                           channel_multiplier=1)
            iota_p = bigp.tile([128, 1], F32)
            nc.vector.tensor_copy(iota_p[:, :], iota_p_i[:, :])
            ltmask = bigp.tile([128, 128], F32)
            nc.vector.tensor_scalar(
                out=ltmask[:, :], in0=iota_row[:, :],
                scalar1=iota_p[:, :1], scalar2=None, op0=alu.is_gt,
            )
            ones_col = bigp.tile([128, 1], F32)
            nc.vector.memset(ones_col[:, :], 1.0)
            ident64 = bigp.tile([64, 64], F32)
            make_identity(nc, ident64[:, :])

            # ---------------- load z (channel-major) and codebook ----------
            zT = bigp.tile([E_DIM, T_LOC], F32)
            nc.sync.dma_start(out=zT[:, :], in_=zT_in[:, :])
            e_sb = bigp.tile([128, 32, E_DIM], F32)
            nc.sync.dma_start(
                out=e_sb[:, :, :],
                in_=w_in[:, :].rearrange("(k p) d -> p k d", p=128),
            )

            # e2T[64, 4096] = 2 * e.T ; ee[4096] = rowwise ||e||^2 (DRAM)
            e2T = bigp.tile([E_DIM, N_E], F32)
            ee_cols = bigp.tile([128, 32], F32)
            scr = bigp.tile([128, E_DIM], F32)
            ee_dram = dram.tile([1, N_E], F32)
            for k in range(32):
                tp = ps_tp.tile([E_DIM, 128], F32, tag="tp")
                nc.tensor.transpose(out=tp[:, :], in_=e_sb[:, k, :],
                                    identity=ident[:, :])
                nc.scalar.mul(e2T[:, k * 128:(k + 1) * 128], tp[:, :], 2.0)
                nc.scalar.activation(
                    scr[:, :], e_sb[:, k, :],
                    mybir.ActivationFunctionType.Square,
                    accum_out=ee_cols[:, k:k + 1],
                )
            nc.sync.dma_start(
                out=ee_dram[0:1, :].rearrange("o (k p) -> (o p) k", p=128),
                in_=ee_cols[:, :],
            )

            nc.sync.dma_start(out=dbg_e2T[:, :], in_=e2T[:, :])
            # zf (token-major) + zz per token
            zf = bigp.tile([128, N_TILES, E_DIM + 1], F32)
            nc.vector.memset(zf[:, :, E_DIM:E_DIM + 1], 1.0)
            zz_all = bigp.tile([128, N_TILES], F32)
            for i in range(N_TILES):
                tp = ps_tp.tile([128, E_DIM], F32, tag="tp")
                nc.tensor.transpose(
                    out=tp[:, :], in_=zT[:, i * 128:(i + 1) * 128],
                    identity=ident64[:, :],
                )
                nc.vector.tensor_copy(zf[:, i, 0:E_DIM], tp[:, :])
                nc.scalar.activation(
                    scr[:, :], zf[:, i, 0:E_DIM],
                    mybir.ActivationFunctionType.Square,
                    accum_out=zz_all[:, i:i + 1],
                )

            # stats accumulator in DRAM (zeroed): rows 0..4095 = [embed_sum |
            # count]; row 4096 col 0 = loss partial
            N_ROT = 8
            stats = dram.tile([STATS_ROWS, 65], F32)
            stats_bufs = []
            for r in range(N_ROT):
                srot = dram.tile([STATS_ROWS, 65], F32, tag=f"srot{r}")
                stats_bufs.append(srot)
            zb = bigp.tile([128, STATS_ROWS * 65 // 128], F32)
            nc.vector.memset(zb[:, :], 0.0)
            znop = nc.gpsimd.engine_nop()
            for sb in stats_bufs:
                zd = nc.sync.dma_start(
                    out=sb[:, :].rearrange("a b -> (a b)")
                                .rearrange("(p k) -> p k", p=128),
                    in_=zb[:, :],
                )
                add_dep_helper(znop.ins, zd.ins, sync=True,
                               reason="gpsimd observes stats zeroing")

            bc_reg = nc.gpsimd.to_reg(N_E - 1)
            idx_all = bigp.tile([128, N_TILES], mybir.dt.int32)
            idx_allf = bigp.tile([128, N_TILES], F32)
            lcols = bigp.tile([128, N_TILES], F32)

            # ---------------- main loop over 64 token tiles ----------------
            for i in range(N_TILES):
                lhsT = zT[:, i * 128:(i + 1) * 128]
                staged = stg.tile([128, N_E + 8], F32, tag="staged")
                nc.vector.memset(staged[:, N_E:N_E + 8], -1e30)
                for base, width in ((0, 1536), (1536, 1536), (3072, 1024)):
                    ps = ps_sc.tile([128, width], F32, tag="sc")
                    for c in range(width // 512):
                        col = base + c * 512
                        nc.tensor.matmul(
                            out=ps[:, c * 512:(c + 1) * 512],
                            lhsT=lhsT,
                            rhs=e2T[:, col:col + 512],
                            start=True, stop=True,
                        )
                    nc.scalar.copy(staged[:, base:base + width],
                                   ps[:, 0:width])

                # top-8 by 2*z@e value, then exact d re-rank
                if i == 32:
                    nc.sync.dma_start(out=dbg_staged[:, :], in_=staged[:, 0:N_E])
                v8 = wrk.tile([128, 8], F32, tag="v8")
                nc.vector.max(out=v8[:, :], in_=staged[:, 0:N_E + 8])
                i8 = wrk.tile([128, 8], mybir.dt.uint32, tag="i8")
                nc.vector.max_index(out=i8[:, :], in_max=v8[:, :],
                                    in_values=staged[:, 0:N_E + 8])
                i8f = wrk.tile([128, 8], F32, tag="i8f")
                nc.vector.tensor_copy(i8f[:, :], i8[:, :])
                i8i = wrk.tile([128, 8], mybir.dt.int32, tag="i8i")
                nc.vector.tensor_copy(i8i[:, :], i8f[:, :])
                ee8 = wrk.tile([128, 8], F32, tag="ee8")
                nc.gpsimd.indirect_dma_start(
                    out=ee8[:, :], out_offset=None,
                    in_=ee_dram[0:1, :].rearrange("o n -> (o n) ()"),
                    in_offset=IndirectOffsetOnAxis(ap=i8i[:, :], axis=0),
                )
                # d8 = fl(fl(zz + ee) - v)   (jax's rounding order)
                a8 = wrk.tile([128, 8], F32, tag="a8")
                nc.vector.tensor_scalar(
                    out=a8[:, :], in0=ee8[:, :],
                    scalar1=zz_all[:, i:i + 1], scalar2=None, op0=alu.add,
                )
                d8 = wrk.tile([128, 8], F32, tag="d8")
                nc.vector.tensor_tensor(out=d8[:, :], in0=a8[:, :],
                                        in1=v8[:, :], op=alu.subtract)
                dmin = wrk.tile([128, 1], F32, tag="dmin")
                nc.vector.tensor_reduce(out=dmin[:, :], in_=d8[:, :],
                                        axis=mybir.AxisListType.X, op=alu.min)
                noteq = wrk.tile([128, 8], F32, tag="noteq")
                nc.vector.tensor_scalar(
                    out=noteq[:, :], in0=d8[:, :],
                    scalar1=dmin[:, :1], scalar2=None, op0=alu.not_equal,
                )
                cand = wrk.tile([128, 8], F32, tag="cand")
                nc.vector.scalar_tensor_tensor(
                    out=cand[:, :], in0=noteq[:, :], scalar=60000.0,
                    in1=i8f[:, :], op0=alu.mult, op1=alu.add,
                )
                idxf = wrk.tile([128, 1], F32, tag="idxf")
                nc.vector.tensor_reduce(out=idxf[:, :], in_=cand[:, :],
                                        axis=mybir.AxisListType.X, op=alu.min)
                nc.vector.tensor_copy(idx_allf[:, i:i + 1], idxf[:, :])
                nc.vector.tensor_copy(idx_all[:, i:i + 1], idxf[:, :])

                # z_q gather, straight-through out, loss contribution
                zq = wrk.tile([128, E_DIM], F32, tag="zq")
                nc.gpsimd.indirect_dma_start(
                    out=zq[:, :], out_offset=None,
                    in_=w_in[:, :],
                    in_offset=IndirectOffsetOnAxis(ap=idx_all[:, i:i + 1], axis=0),
                )
                dlt = wrk.tile([128, E_DIM], F32, tag="dlt")
                nc.vector.tensor_tensor(out=dlt[:, :], in0=zq[:, :],
                                        in1=zf[:, i, 0:E_DIM], op=alu.subtract)
                nc.scalar.activation(
                    scr[:, :], dlt[:, :],
                    mybir.ActivationFunctionType.Square,
                    accum_out=lcols[:, i:i + 1],
                )

                # in-tile dedup for the scatter: S = (idx == idx.T)
                tp = ps_tp.tile([128, 128], F32, tag="tp")
                nc.tensor.transpose(
                    out=tp[:, :],
                    in_=idx_allf[:, i:i + 1].to_broadcast([128, 128]),
                    identity=ident[:, :],
                )
                idxT = wrk.tile([128, 128], F32, tag="idxT")
                nc.vector.tensor_copy(idxT[:, :], tp[:, :])
                S = wrk.tile([128, 128], F32, tag="S")
                nc.vector.tensor_tensor(
                    out=S[:, :],
                    in0=idx_allf[:, i:i + 1].to_broadcast([128, 128]),
                    in1=idxT[:, :], op=alu.is_equal,
                )
                comb_ps = ps_tp.tile([128, 65], F32, tag="comb")
                nc.tensor.matmul(out=comb_ps[:, :], lhsT=S[:, :],
                                 rhs=zf[:, i, :], start=True, stop=True)
                comb = wrk.tile([128, 65], F32, tag="comb_sb")
                nc.vector.tensor_copy(comb[:, :], comb_ps[:, :])
                SL = wrk.tile([128, 128], F32, tag="SL")
                nc.vector.tensor_tensor(out=SL[:, :], in0=S[:, :],
                                        in1=ltmask[:, :], op=alu.mult)
                cnt = wrk.tile([128, 1], F32, tag="cnt")
                nc.vector.tensor_reduce(out=cnt[:, :], in_=SL[:, :],
                                        axis=mybir.AxisListType.X, op=alu.add)
                notfirst = wrk.tile([128, 1], F32, tag="nf")
                nc.vector.tensor_scalar(
                    out=notfirst[:, :], in0=cnt[:, :],
                    scalar1=0.0, scalar2=None, op0=alu.is_gt,
                )
                idx_eff_f = wrk.tile([128, 1], F32, tag="ieff_f")
                nc.vector.scalar_tensor_tensor(
                    out=idx_eff_f[:, :], in0=notfirst[:, :], scalar=60000.0,
                    in1=idx_allf[:, i:i + 1], op0=alu.mult, op1=alu.add,
                )
                idx_eff = wrk.tile([128, 1], mybir.dt.int32, tag="ieff")
                nc.vector.tensor_copy(idx_eff[:, :], idx_eff_f[:, :])
                sc_dma = nc.gpsimd.indirect_dma_start(
                    out=stats_bufs[i % N_ROT][:, :],
                    out_offset=IndirectOffsetOnAxis(ap=idx_eff[:, :1], axis=0),
                    in_=comb[:, :], in_offset=None,
                    compute_op=alu.add,
                    bounds_check=bc_reg, oob_is_err=False,
                )
                add_dep_helper(sc_dma.ins, znop.ins, sync=False,
                               reason="scatter after stats zeroing")

                # straight-through output overwrites zf slot:
                # out = zp + (z_q - zp), computed exactly like jax
                nc.vector.tensor_tensor(out=zf[:, i, 0:E_DIM],
                                        in0=zf[:, i, 0:E_DIM],
                                        in1=dlt[:, :], op=alu.add)

            # idx output
            nc.sync.dma_start(out=idx_out[:, :], in_=idx_all[:, :])

            # ---------------- loss partial -> stats row 4096 ---------------
            lsum = bigp.tile([128, 1], F32)
            nc.vector.tensor_reduce(out=lsum[:, :], in_=lcols[:, :],
                                    axis=mybir.AxisListType.X, op=alu.add)
            lt_ps = ps_tp.tile([1, 1], F32, tag="comb")
            nc.tensor.matmul(out=lt_ps[:, :], lhsT=ones_col[:, :],
                             rhs=lsum[:, :], start=True, stop=True)
            lpart = bigp.tile([1, 1], F32)
            nc.vector.tensor_copy(lpart[:, :], lt_ps[:, :])
            lp_dma = nc.sync.dma_start(out=stats_bufs[0][4096:4097, 0:1],
                                       in_=lpart[:, :])
            add_dep_helper(lp_dma.ins, znop.ins, sync=False,
                           reason="partial write after zeroing")
            # merge the rotated partials into the collective input
            FLAT = STATS_ROWS * 65 // 128
            macc = bigp.tile([128, FLAT], F32)
            mtmp = zb  # zeroing source is dead by now; reuse as merge temp
            nc.sync.dma_start(
                out=macc[:, :],
                in_=stats_bufs[0][:, :].rearrange("a b -> (a b)")
                                       .rearrange("(p k) -> p k", p=128))
            for r in range(1, N_ROT):
                nc.sync.dma_start(
                    out=mtmp[:, :],
                    in_=stats_bufs[r][:, :].rearrange("a b -> (a b)")
                                           .rearrange("(p k) -> p k", p=128))
                nc.vector.tensor_tensor(out=macc[:, :], in0=macc[:, :],
                                        in1=mtmp[:, :], op=alu.add)
            nc.sync.dma_start(
                out=stats[:, :].rearrange("a b -> (a b)")
                               .rearrange("(p k) -> p k", p=128),
                in_=macc[:, :])

            # ---------------- all-reduce the stats ----------------
            stats_red = dram.tile([STATS_ROWS, 65], F32)
            cc = nc.gpsimd.collective_compute(
                "AllReduce", alu.add,
                replica_groups=[list(range(N_CORES))],
                ins=[stats[:, :]], outs=[stats_red[:, :]],
            )

            # ---------------- EMA update + scalars (redundant per core) ----
            es = bigp.tile([128, 32, E_DIM], F32)
            nc.sync.dma_start(
                out=es[:, :, :],
                in_=stats_red[0:N_E, 0:E_DIM].rearrange(
                    "(k p) d -> p k d", p=128),
            )
            counts = bigp.tile([128, 32], F32)
            nc.sync.dma_start(
                out=counts[:, :],
                in_=stats_red[0:N_E, E_DIM:E_DIM + 1].rearrange(
                    "(k p) o -> p (k o)", p=128),
            )
            cs_sb = bigp.tile([128, 32], F32)
            nc.sync.dma_start(
                out=cs_sb[:, :],
                in_=cs_in[:].rearrange("(k p) -> p k", p=128),
            )
            ea_sb = bigp.tile([128, 32, E_DIM], F32)
            nc.sync.dma_start(
                out=ea_sb[:, :, :],
                in_=ea_in[:, :].rearrange("(k p) d -> p k d", p=128),
            )

            # new_cs = fl(cs*0.99) + fl(counts*0.01)
            cs99 = bigp.tile([128, 32], F32)
            nc.vector.tensor_scalar_mul(cs99[:, :], cs_sb[:, :], DECAY)
            new_cs = bigp.tile([128, 32], F32)
            nc.vector.scalar_tensor_tensor(
                out=new_cs[:, :], in0=counts[:, :], scalar=1.0 - DECAY,
                in1=cs99[:, :], op0=alu.mult, op1=alu.add,
            )
            # new_ea = fl(ea*0.99) + fl(es*0.01)  (overwrite ea_sb)
            nc.vector.tensor_scalar_mul(ea_sb[:, :, :], ea_sb[:, :, :], DECAY)
            new_ea = bigp.tile([128, 32, E_DIM], F32)
            nc.vector.scalar_tensor_tensor(
                out=new_ea[:, :, :], in0=es[:, :, :], scalar=1.0 - DECAY,
                in1=ea_sb[:, :, :], op0=alu.mult, op1=alu.add,
            )

            # n = sum(new_cs) -> broadcast to all partitions via PE
            ncs_sum = bigp.tile([1, 32], F32)
            s_ps = ps_tp.tile([1, 32], F32, tag="comb")
            nc.tensor.matmul(out=s_ps[:, :], lhsT=ones_col[:, :],
                             rhs=new_cs[:, :], start=True, stop=True)
            nc.vector.tensor_copy(ncs_sum[:, :], s_ps[:, :])
            n_11 = bigp.tile([1, 1], F32)
            nc.vector.tensor_reduce(out=n_11[:, :], in_=ncs_sum[:, :],
                                    axis=mybir.AxisListType.X, op=alu.add)
            nb_ps = ps_tp.tile([128, 1], F32, tag="comb")
            nc.tensor.matmul(out=nb_ps[:, :],
                             lhsT=ones_col[0:1, :].to_broadcast([1, 128]),
                             rhs=n_11[:, :], start=True, stop=True)
            n_bc = bigp.tile([128, 1], F32)
            nc.vector.tensor_copy(n_bc[:, :], nb_ps[:, :])

            # smoothed = (new_cs + eps) / (n + N_E*eps) * n
            denom = bigp.tile([128, 1], F32)
            nc.vector.tensor_scalar_add(denom[:, :], n_bc[:, :], N_E * EPS)
            rden = bigp.tile([128, 1], F32)
            nc.vector.reciprocal(rden[:, :], denom[:, :])
            smf = bigp.tile([128, 32], F32)
            nc.vector.tensor_scalar(
                out=smf[:, :], in0=new_cs[:, :],
                scalar1=EPS, scalar2=rden[:, :1], op0=alu.add, op1=alu.mult,
            )
            sm = bigp.tile([128, 32], F32)
            nc.vector.tensor_scalar(
                out=sm[:, :], in0=smf[:, :],
                scalar1=n_bc[:, :1], scalar2=None, op0=alu.mult,
            )
            rsm = bigp.tile([128, 32], F32)
            nc.vector.reciprocal(rsm[:, :], sm[:, :])
            new_embed = es  # es fully consumed; reuse the buffer
            nc.vector.tensor_tensor(
                out=new_embed[:, :, :], in0=new_ea[:, :, :],
                in1=rsm[:, :].rearrange("p k -> p k ()").to_broadcast(
                    [128, 32, E_DIM]),
                op=alu.mult,
            )

            # perplexity = exp(-sum(p*log(p+1e-10))), p = counts/65536
            avg = bigp.tile([128, 32], F32)
            nc.vector.tensor_scalar_mul(avg[:, :], counts[:, :], 1.0 / N_TOK)
            avg_e = bigp.tile([128, 32], F32)
            nc.vector.tensor_scalar_add(avg_e[:, :], avg[:, :], 1e-10)
            lg = bigp.tile([128, 32], F32)
            nc.scalar.activation(lg[:, :], avg_e[:, :],
                                 mybir.ActivationFunctionType.Ln)
            pl = bigp.tile([128, 32], F32)
            nc.vector.tensor_tensor(out=pl[:, :], in0=avg[:, :], in1=lg[:, :],
                                    op=alu.mult)
            pls = bigp.tile([128, 1], F32)
            nc.vector.tensor_reduce(out=pls[:, :], in_=pl[:, :],
                                    axis=mybir.AxisListType.X, op=alu.add)
            e_ps = ps_tp.tile([1, 1], F32, tag="comb")
            nc.tensor.matmul(out=e_ps[:, :], lhsT=ones_col[:, :],
                             rhs=pls[:, :], start=True, stop=True)
            ent = bigp.tile([1, 1], F32)
            nc.vector.tensor_scalar_mul(ent[:, :], e_ps[:, :], -1.0)
            ppl = bigp.tile([1, 1], F32)
            nc.scalar.activation(ppl[:, :], ent[:, :],
                                 mybir.ActivationFunctionType.Exp)
            nc.sync.dma_start(out=ppl_out[:, :], in_=ppl[:, :])

            # loss = BETA * loss_total / N
            lt = bigp.tile([1, 1], F32)
            nc.sync.dma_start(out=lt[:, :], in_=stats_red[4096:4097, 0:1])
            lossv = bigp.tile([1, 1], F32)
            nc.vector.tensor_scalar(
                out=lossv[:, :], in0=lt[:, :],
                scalar1=1.0 / (N_TOK * E_DIM), scalar2=BETA,
                op0=alu.mult, op1=alu.mult,
            )
            nc.sync.dma_start(out=loss_out[:, :], in_=lossv[:, :])

            # EMA outputs
            nc.sync.dma_start(
                out=nemb_out[:, :].rearrange("(k p) d -> p k d", p=128),
                in_=new_embed[:, :, :],
            )
            nc.sync.dma_start(
                out=ncs_out[:].rearrange("(k p) -> p k", p=128),
                in_=new_cs[:, :],
            )
            nc.sync.dma_start(
                out=nea_out[:, :].rearrange("(k p) d -> p k d", p=128),
                in_=new_ea[:, :, :],
            )

            # ---------------- straight-through output ----------------
            outT = bigp.tile([E_DIM, T_LOC], F32)
            for i in range(N_TILES):
                tp = ps_tp.tile([E_DIM, 128], F32, tag="tp")
                nc.tensor.transpose(out=tp[:, :], in_=zf[:, i, 0:E_DIM],
                                    identity=ident[:, :])
                nc.scalar.copy(outT[:, i * 128:(i + 1) * 128], tp[:, :])
            nc.sync.dma_start(out=out_sh[:, :], in_=outT[:, :])

    _split_excess_waits(nc)
    return nc


_NC_CACHE = []


def kernel(z, embed_w, cluster_size, embed_avg):
    z = np.ascontiguousarray(z, dtype=np.float32)
    embed_w = np.ascontiguousarray(embed_w, dtype=np.float32)
    cluster_size = np.ascontiguousarray(cluster_size, dtype=np.float32)
    embed_avg = np.ascontiguousarray(embed_avg, dtype=np.float32)

    if not _NC_CACHE:
        _NC_CACHE.append(build_nc())
    nc = _NC_CACHE[0]

    zr = z.reshape(4, 64, 16384)
    in_maps = []
    for c in range(N_CORES):
        b, half = c // 2, c % 2
        zT_c = np.ascontiguousarray(zr[b, :, half * T_LOC:(half + 1) * T_LOC])
        in_maps.append({
            "zT": zT_c,
            "embed_w": embed_w,
            "cluster_size": cluster_size,
            "embed_avg": embed_avg,
        })

    res = run_bass_kernel_spmd(nc, in_maps, core_ids=list(range(N_CORES)))
    rs = res.results

    out = np.empty((4, 64, 16384), dtype=np.float32)
    idx = np.empty((N_CORES, T_LOC), dtype=np.int32)
    for c in range(N_CORES):
        b, half = c // 2, c % 2
        out[b, :, half * T_LOC:(half + 1) * T_LOC] = rs[c]["out_sh"]
        idx[c] = rs[c]["idx_t"].T.reshape(T_LOC)
    out = out.reshape(4, 64, 16, 32, 32)
    idx = idx.reshape(N_TOK)

    r0 = rs[0]
    loss = np.float32(r0["loss"][0, 0])
    ppl = np.float32(r0["ppl"][0, 0])
    return (out, loss, ppl, idx, r0["new_embed"], r0["new_cs"], r0["new_ea"])


# revision 12
# speedup vs baseline: 1.0377x; 1.0377x over previous
"""EMA VectorQuantizer forward pass on 8 TRN2 NeuronCores (Bass/Tile).

Data-parallel over tokens: z [4,64,16,32,32] -> 65536 tokens of dim 64,
8192 tokens per core (channel-major shard [64, 8192] is a natural slice
of z's layout). The [4096,64] codebook is replicated. Per core:
  scores = 2*z@e.T via PE fp32 matmuls (tokens on partitions, codes on
  free dim), top-8 via DVE max8/max_index, then an exact fp32 re-ranking
  of the 8 candidates replicating jax's rounding of
  d = (||z||^2 + ||e||^2) - 2*z@e.T (the (zz+ee) double-rounding decides
  ~3% of tokens' grid-ties, so it must be emulated bit-exactly).
  z_q by indirect row gather; counts/embed_sum by per-tile dedup
  (selection-matrix matmul) + serialized indirect scatter-add DMAs; the
  per-core partial stats are AllReduce'd and the EMA buffer update is
  computed redundantly on every core.
"""

import numpy as np

import bass_rust
import concourse.bass as bass
import concourse.mybir as mybir
import concourse.tile as tile_mod
from concourse.bass import IndirectOffsetOnAxis
from concourse.masks import make_identity
from concourse.tile import TileContext
from concourse.tile_rust import add_dep_helper
from concourse.bass_utils import run_bass_kernel_spmd

N_CORES = 8
N_E = 4096
E_DIM = 64
T_LOC = 8192          # tokens per core
N_TILES = T_LOC // 128
BETA = 0.25
DECAY = 0.99
EPS = 1e-05
N_TOK = 65536
STATS_ROWS = 4224     # 4096 codes + spare rows for scalar partials
F32 = mybir.dt.float32

# ---------------------------------------------------------------------------
# workaround 1: walrus in this container rejects >1 sem wait on the
# TileContext tail drain — pre-absorb the global-clock waits one per drain.

def _patched_drain_and_barrier(self, tick_clock, wait_clock):
    nc = self.nc
    vc = tick_clock.global_clock
    nonzero = [(i, vc[i]) for i in range(len(vc)) if vc[i] > 0]
    for i, t in nonzero:
        pvc = bass_rust.VectorClock([0] * len(vc))
        pvc.require_at_least(i, t)
        nop = nc.sync.drain()
        wait_clock.add_sem_waits(nop.ins, bass_rust.ScopedClock({None: pvc}))
    nc.sync.drain()
    nc.all_engine_barrier()
    assert self.sems is not None
    popped = nc._tile_sem_poison_stack.pop()
    assert popped is self._sem_poison
    nc.clear_and_free_semaphores(list(self.sems.allocated().values()))
    nc.all_engine_barrier()


tile_mod.TileContext._drain_and_barrier = _patched_drain_and_barrier

# ---------------------------------------------------------------------------
# workaround 2: same walrus cap on every other instruction — hoist excess
# semaphore waits onto same-engine NoOps inserted right before it.

_wsplit_ctr = [0]


def _split_excess_waits(nc, max_sem_waits=1):
    for f in nc.m.functions:
        for bb in f.blocks:
            insts = bb.instructions
            new = []
            changed = False
            for inst in insts:
                si = inst.sync_info
                waits = list(si.on_wait) if (si and si.on_wait) else []
                sem_w = [w for w in waits if w.sync_type == "semaphore"]
                other_w = [w for w in waits if w.sync_type != "semaphore"]
                keep = max(0, max_sem_waits - len(other_w))
                if len(sem_w) > keep:
                    excess = sem_w[: len(sem_w) - keep]
                    kept = sem_w[len(sem_w) - keep:]
                    for w in excess:
                        _wsplit_ctr[0] += 1
                        nop = mybir.InstNoOp(
                            name=f"I-wsplit-{_wsplit_ctr[0]}", ins=[], outs=[]
                        )
                        nop.engine = inst.engine
                        nop.sync_info = mybir.SyncInfo(on_wait=[w], on_update=[])
                        new.append(nop)
                    inst.sync_info = mybir.SyncInfo(
                        on_wait=other_w + kept,
                        on_update=list(si.on_update) if si.on_update else [],
                    )
                    changed = True
                new.append(inst)
            if changed:
                insts[:] = new


# ---------------------------------------------------------------------------

def build_nc():
    nc = bass.Bass(trn_type="TRN2", num_devices=N_CORES)
    alu = mybir.AluOpType

    zT_in = nc.dram_tensor("zT", [E_DIM, T_LOC], F32, kind="ExternalInput")
    w_in = nc.dram_tensor("embed_w", [N_E, E_DIM], F32, kind="ExternalInput")
    cs_in = nc.dram_tensor("cluster_size", [N_E], F32, kind="ExternalInput")
    ea_in = nc.dram_tensor("embed_avg", [N_E, E_DIM], F32, kind="ExternalInput")

    out_sh = nc.dram_tensor("out_sh", [E_DIM, T_LOC], F32, kind="ExternalOutput")
    idx_out = nc.dram_tensor("idx_t", [128, N_TILES], mybir.dt.int32,
                             kind="ExternalOutput")
    loss_out = nc.dram_tensor("loss", [1, 1], F32, kind="ExternalOutput")
    ppl_out = nc.dram_tensor("ppl", [1, 1], F32, kind="ExternalOutput")
    nemb_out = nc.dram_tensor("new_embed", [N_E, E_DIM], F32, kind="ExternalOutput")
    dbg_staged = nc.dram_tensor("dbg_staged", [128, N_E], F32, kind="ExternalOutput")
    dbg_e2T = nc.dram_tensor("dbg_e2T", [E_DIM, N_E], F32, kind="ExternalOutput")
    dbg_ref = nc.dram_tensor("dbg_ref", [128, 40], F32, kind="ExternalOutput")
    ncs_out = nc.dram_tensor("new_cs", [N_E], F32, kind="ExternalOutput")
    nea_out = nc.dram_tensor("new_ea", [N_E, E_DIM], F32, kind="ExternalOutput")

    with TileContext(nc) as tc:
        with (
            tc.tile_pool(name="big", bufs=1) as bigp,      # persistent SBUF
            tc.tile_pool(name="wrk", bufs=2) as wrk,       # per-tile rotating
            tc.tile_pool(name="stg", bufs=2) as stg,       # staged scores
            tc.tile_pool(name="ps_sc", bufs=2, space="PSUM") as ps_sc,
            tc.tile_pool(name="ps_tp", bufs=1, space="PSUM") as ps_tp,
            tc.tile_pool(name="dram", bufs=1, space="DRAM") as dram,
        ):
            # ---------------- constants / setup ----------------
            ident = bigp.tile([128, 128], F32)
            make_identity(nc, ident[:, :])
            iota_row_i = bigp.tile([128, 128], mybir.dt.int32)
            nc.gpsimd.iota(iota_row_i[:, :], pattern=[[1, 128]], base=0,
                           channel_multiplier=0)
            iota_row = bigp.tile([128, 128], F32)
            nc.vector.tensor_copy(iota_row[:, :], iota_row_i[:, :])
            iota_p_i = bigp.tile([128, 1], mybir.dt.int32)
            nc.gpsimd.iota(iota_p_i[:, :], pattern=[[1, 1]], base=0,
                           channel_multiplier=1)
            iota_p = bigp.tile([128, 1], F32)
            nc.vector.tensor_copy(iota_p[:, :], iota_p_i[:, :])
            ltmask = bigp.tile([128, 128], F32)
            nc.vector.tensor_scalar(
                out=ltmask[:, :], in0=iota_row[:, :],
                scalar1=iota_p[:, :1], scalar2=None, op0=alu.is_gt,
            )
            ones_col = bigp.tile([128, 1], F32)
            nc.vector.memset(ones_col[:, :], 1.0)
            ident64 = bigp.tile([64, 64], F32)
            make_identity(nc, ident64[:, :])

            # ---------------- load z (channel-major) and codebook ----------
            zT = bigp.tile([E_DIM, T_LOC], F32, tag="zT_slot")
            nc.sync.dma_start(out=zT[:, :], in_=zT_in[:, :])
            e_sb = bigp.tile([128, 32, E_DIM], F32)
            nc.sync.dma_start(
                out=e_sb[:, :, :],
                in_=w_in[:, :].rearrange("(k p) d -> p k d", p=128),
            )

            # e2T[64, 4096] = 2 * e.T ; ee[4096] = rowwise ||e||^2 (DRAM)
            e2T = bigp.tile([E_DIM, N_E], F32)
            ee_cols = bigp.tile([128, 32], F32)
            scr = bigp.tile([128, E_DIM], F32)
            ee_dram = dram.tile([1, N_E], F32)
            for k in range(32):
                tp = ps_tp.tile([E_DIM, 128], F32, tag="tp")
                nc.tensor.transpose(out=tp[:, :], in_=e_sb[:, k, :],
                                    identity=ident[:, :])
                nc.scalar.mul(e2T[:, k * 128:(k + 1) * 128], tp[:, :], 2.0)
                nc.scalar.activation(
                    scr[:, :], e_sb[:, k, :],
                    mybir.ActivationFunctionType.Square,
                    accum_out=ee_cols[:, k:k + 1],
                )
            nc.sync.dma_start(
                out=ee_dram[0:1, :].rearrange("o (k p) -> (o p) k", p=128),
                in_=ee_cols[:, :],
            )
            negEE = bigp.tile([128, N_E], F32)
            nc.sync.dma_start(
                out=negEE[:, :],
                in_=ee_dram[0:1, :].to_broadcast([128, N_E]),
            )
            nc.vector.tensor_scalar_mul(negEE[:, :], negEE[:, :], -1.0)

            nc.sync.dma_start(out=dbg_e2T[:, :], in_=e2T[:, :])
            # zf (token-major) + zz per token
            zf = bigp.tile([128, N_TILES, E_DIM + 1], F32)
            nc.vector.memset(zf[:, :, E_DIM:E_DIM + 1], 1.0)
            zz_all = bigp.tile([128, N_TILES], F32)
            for i in range(N_TILES):
                tp = ps_tp.tile([128, E_DIM], F32, tag="tp")
                nc.tensor.transpose(
                    out=tp[:, :], in_=zT[:, i * 128:(i + 1) * 128],
                    identity=ident64[:, :],
                )
                nc.vector.tensor_copy(zf[:, i, 0:E_DIM], tp[:, :])
                nc.scalar.activation(
                    scr[:, :], zf[:, i, 0:E_DIM],
                    mybir.ActivationFunctionType.Square,
                    accum_out=zz_all[:, i:i + 1],
                )

            # stats accumulator in DRAM (zeroed): rows 0..4095 = [embed_sum |
            # count]; row 4096 col 0 = loss partial
            negzz_all = bigp.tile([128, N_TILES], F32)
            nc.vector.tensor_scalar_mul(negzz_all[:, :], zz_all[:, :], -1.0)
            N_ROT = 8
            stats = dram.tile([STATS_ROWS, 65], F32)
            stats_bufs = []
            for r in range(N_ROT):
                srot = dram.tile([STATS_ROWS, 65], F32, tag=f"srot{r}")
                stats_bufs.append(srot)
            zb = bigp.tile([128, STATS_ROWS * 65 // 128], F32)
            nc.vector.memset(zb[:, :], 0.0)
            znop = nc.gpsimd.engine_nop()
            for sb in stats_bufs:
                zd = nc.sync.dma_start(
                    out=sb[:, :].rearrange("a b -> (a b)")
                                .rearrange("(p k) -> p k", p=128),
                    in_=zb[:, :],
                )
                add_dep_helper(znop.ins, zd.ins, sync=True,
                               reason="gpsimd observes stats zeroing")

            bc_reg = nc.gpsimd.to_reg(N_E - 1)
            idx_all = bigp.tile([128, N_TILES], mybir.dt.int32)
            idx_allf = bigp.tile([128, N_TILES], F32)
            lcols = bigp.tile([128, N_TILES], F32)
            comb_all = bigp.tile([128, N_TILES, 65], F32)
            ieff_all = bigp.tile([128, N_TILES], mybir.dt.int32)

            # ---------------- main loop over 64 token tiles ----------------
            for i in range(N_TILES):
                lhsT = zT[:, i * 128:(i + 1) * 128]
                staged = stg.tile([128, N_E + 8], F32, tag="staged")
                nc.vector.memset(staged[:, N_E:N_E + 8], -1e30)
                for base, width in ((0, 1536), (1536, 1536), (3072, 1024)):
                    ps = ps_sc.tile([128, width], F32, tag="sc")
                    for c in range(width // 512):
                        col = base + c * 512
                        nc.tensor.matmul(
                            out=ps[:, c * 512:(c + 1) * 512],
                            lhsT=lhsT,
                            rhs=e2T[:, col:col + 512],
                            start=True, stop=True,
                        )
                    # staged = fl(fl(-ee - zz) + 2*z@e) = -d with jax's
                    # exact double rounding; max over it = argmin of d
                    nc.vector.scalar_tensor_tensor(
                        out=staged[:, base:base + width],
                        in0=negEE[:, base:base + width],
                        scalar=negzz_all[:, i:i + 1],
                        in1=ps[:, 0:width],
                        op0=alu.add, op1=alu.add,
                    )

                if i == 32:
                    nc.sync.dma_start(out=dbg_staged[:, :], in_=staged[:, 0:N_E])
                v8 = wrk.tile([128, 8], F32, tag="v8")
                nc.vector.max(out=v8[:, :], in_=staged[:, 0:N_E + 8])
                i8 = wrk.tile([128, 8], mybir.dt.uint32, tag="i8")
                nc.vector.max_index(out=i8[:, :], in_max=v8[:, :],
                                    in_values=staged[:, 0:N_E + 8])
                i8f = wrk.tile([128, 8], F32, tag="i8f")
                nc.vector.tensor_copy(i8f[:, 0:1], i8[:, 0:1])
                nc.vector.tensor_copy(idx_allf[:, i:i + 1], i8f[:, 0:1])
                nc.vector.tensor_copy(idx_all[:, i:i + 1], i8f[:, 0:1])

                # z_q gather, straight-through out, loss contribution
                zq = wrk.tile([128, E_DIM], F32, tag="zq")
                nc.gpsimd.indirect_dma_start(
                    out=zq[:, :], out_offset=None,
                    in_=w_in[:, :],
                    in_offset=IndirectOffsetOnAxis(ap=idx_all[:, i:i + 1], axis=0),
                )
                dlt = wrk.tile([128, E_DIM], F32, tag="dlt")
                nc.vector.tensor_tensor(out=dlt[:, :], in0=zq[:, :],
                                        in1=zf[:, i, 0:E_DIM], op=alu.subtract)
                nc.scalar.activation(
                    scr[:, :], dlt[:, :],
                    mybir.ActivationFunctionType.Square,
                    accum_out=lcols[:, i:i + 1],
                )

                # in-tile dedup for the scatter: S = (idx == idx.T)
                tp = ps_tp.tile([128, 128], F32, tag="tp")
                nc.tensor.transpose(
                    out=tp[:, :],
                    in_=idx_allf[:, i:i + 1].to_broadcast([128, 128]),
                    identity=ident[:, :],
                )
                idxT = wrk.tile([128, 128], F32, tag="idxT")
                nc.vector.tensor_copy(idxT[:, :], tp[:, :])
                S = wrk.tile([128, 128], F32, tag="S")
                nc.vector.tensor_tensor(
                    out=S[:, :],
                    in0=idx_allf[:, i:i + 1].to_broadcast([128, 128]),
                    in1=idxT[:, :], op=alu.is_equal,
                )
                comb_ps = ps_tp.tile([128, 65], F32, tag="comb")
                nc.tensor.matmul(out=comb_ps[:, :], lhsT=S[:, :],
                                 rhs=zf[:, i, :], start=True, stop=True)
                nc.vector.tensor_copy(comb_all[:, i, :], comb_ps[:, :])
                SL = wrk.tile([128, 128], F32, tag="SL")
                nc.vector.tensor_tensor(out=SL[:, :], in0=S[:, :],
                                        in1=ltmask[:, :], op=alu.mult)
                cnt = wrk.tile([128, 1], F32, tag="cnt")
                nc.vector.tensor_reduce(out=cnt[:, :], in_=SL[:, :],
                                        axis=mybir.AxisListType.X, op=alu.add)
                notfirst = wrk.tile([128, 1], F32, tag="nf")
                nc.vector.tensor_scalar(
                    out=notfirst[:, :], in0=cnt[:, :],
                    scalar1=0.0, scalar2=None, op0=alu.is_gt,
                )
                idx_eff_f = wrk.tile([128, 1], F32, tag="ieff_f")
                nc.vector.scalar_tensor_tensor(
                    out=idx_eff_f[:, :], in0=notfirst[:, :], scalar=60000.0,
                    in1=idx_allf[:, i:i + 1], op0=alu.mult, op1=alu.add,
                )
                nc.vector.tensor_copy(ieff_all[:, i:i + 1], idx_eff_f[:, :])
                sc_dma = nc.gpsimd.indirect_dma_start(
                    out=stats_bufs[i % N_ROT][:, :],
                    out_offset=IndirectOffsetOnAxis(ap=ieff_all[:, i:i + 1], axis=0),
                    in_=comb_all[:, i, :], in_offset=None,
                    compute_op=alu.add,
                    bounds_check=bc_reg, oob_is_err=False,
                )
                add_dep_helper(sc_dma.ins, znop.ins, sync=False,
                               reason="scatter after stats zeroing")

                # straight-through output overwrites zf slot:
                # out = zp + (z_q - zp), computed exactly like jax
                nc.vector.tensor_tensor(out=zf[:, i, 0:E_DIM],
                                        in0=zf[:, i, 0:E_DIM],
                                        in1=dlt[:, :], op=alu.add)

            # idx output
            nc.sync.dma_start(out=idx_out[:, :], in_=idx_all[:, :])

            # ---------------- loss partial -> stats row 4096 ---------------
            lsum = bigp.tile([128, 1], F32)
            nc.vector.tensor_reduce(out=lsum[:, :], in_=lcols[:, :],
                                    axis=mybir.AxisListType.X, op=alu.add)
            lt_ps = ps_tp.tile([1, 1], F32, tag="comb")
            nc.tensor.matmul(out=lt_ps[:, :], lhsT=ones_col[:, :],
                             rhs=lsum[:, :], start=True, stop=True)
            lpart = bigp.tile([1, 1], F32)
            nc.vector.tensor_copy(lpart[:, :], lt_ps[:, :])
            lp_dma = nc.sync.dma_start(out=stats_bufs[0][4096:4097, 0:1],
                                       in_=lpart[:, :])
            add_dep_helper(lp_dma.ins, znop.ins, sync=False,
                           reason="partial write after zeroing")
            # merge the rotated partials into the collective input
            FLAT = STATS_ROWS * 65 // 128
            macc = bigp.tile([128, FLAT], F32)
            mtmp = zb  # zeroing source is dead by now; reuse as merge temp
            nc.sync.dma_start(
                out=macc[:, :],
                in_=stats_bufs[0][:, :].rearrange("a b -> (a b)")
                                       .rearrange("(p k) -> p k", p=128))
            for r in range(1, N_ROT):
                nc.sync.dma_start(
                    out=mtmp[:, :],
                    in_=stats_bufs[r][:, :].rearrange("a b -> (a b)")
                                           .rearrange("(p k) -> p k", p=128))
                nc.vector.tensor_tensor(out=macc[:, :], in0=macc[:, :],
                                        in1=mtmp[:, :], op=alu.add)
            nc.sync.dma_start(
                out=stats[:, :].rearrange("a b -> (a b)")
                               .rearrange("(p k) -> p k", p=128),
                in_=macc[:, :])

            # ---------------- all-reduce the stats ----------------
            stats_red = dram.tile([STATS_ROWS, 65], F32)
            cc = nc.gpsimd.collective_compute(
                "AllReduce", alu.add,
                replica_groups=[list(range(N_CORES))],
                ins=[stats[:, :]], outs=[stats_red[:, :]],
            )

            # ---------------- EMA update + scalars (redundant per core) ----
            es = bigp.tile([128, 32, E_DIM], F32)
            nc.sync.dma_start(
                out=es[:, :, :],
                in_=stats_red[0:N_E, 0:E_DIM].rearrange(
                    "(k p) d -> p k d", p=128),
            )
            counts = bigp.tile([128, 32], F32)
            nc.sync.dma_start(
                out=counts[:, :],
                in_=stats_red[0:N_E, E_DIM:E_DIM + 1].rearrange(
                    "(k p) o -> p (k o)", p=128),
            )
            cs_sb = bigp.tile([128, 32], F32)
            nc.sync.dma_start(
                out=cs_sb[:, :],
                in_=cs_in[:].rearrange("(k p) -> p k", p=128),
            )
            ea_sb = bigp.tile([128, 32, E_DIM], F32)
            nc.sync.dma_start(
                out=ea_sb[:, :, :],
                in_=ea_in[:, :].rearrange("(k p) d -> p k d", p=128),
            )

            # new_cs = fl(cs*0.99) + fl(counts*0.01)
            cs99 = bigp.tile([128, 32], F32)
            nc.vector.tensor_scalar_mul(cs99[:, :], cs_sb[:, :], DECAY)
            new_cs = bigp.tile([128, 32], F32)
            nc.vector.scalar_tensor_tensor(
                out=new_cs[:, :], in0=counts[:, :], scalar=1.0 - DECAY,
                in1=cs99[:, :], op0=alu.mult, op1=alu.add,
            )
            # new_ea = fl(ea*0.99) + fl(es*0.01)  (overwrite ea_sb)
            nc.vector.tensor_scalar_mul(ea_sb[:, :, :], ea_sb[:, :, :], DECAY)
            new_ea = bigp.tile([128, 32, E_DIM], F32)
            nc.vector.scalar_tensor_tensor(
                out=new_ea[:, :, :], in0=es[:, :, :], scalar=1.0 - DECAY,
                in1=ea_sb[:, :, :], op0=alu.mult, op1=alu.add,
            )

            # n = sum(new_cs) -> broadcast to all partitions via PE
            ncs_sum = bigp.tile([1, 32], F32)
            s_ps = ps_tp.tile([1, 32], F32, tag="comb")
            nc.tensor.matmul(out=s_ps[:, :], lhsT=ones_col[:, :],
                             rhs=new_cs[:, :], start=True, stop=True)
            nc.vector.tensor_copy(ncs_sum[:, :], s_ps[:, :])
            n_11 = bigp.tile([1, 1], F32)
            nc.vector.tensor_reduce(out=n_11[:, :], in_=ncs_sum[:, :],
                                    axis=mybir.AxisListType.X, op=alu.add)
            nb_ps = ps_tp.tile([128, 1], F32, tag="comb")
            nc.tensor.matmul(out=nb_ps[:, :],
                             lhsT=ones_col[0:1, :].to_broadcast([1, 128]),
                             rhs=n_11[:, :], start=True, stop=True)
            n_bc = bigp.tile([128, 1], F32)
            nc.vector.tensor_copy(n_bc[:, :], nb_ps[:, :])

            # smoothed = (new_cs + eps) / (n + N_E*eps) * n
            denom = bigp.tile([128, 1], F32)
            nc.vector.tensor_scalar_add(denom[:, :], n_bc[:, :], N_E * EPS)
            rden = bigp.tile([128, 1], F32)
            nc.vector.reciprocal(rden[:, :], denom[:, :])
            smf = bigp.tile([128, 32], F32)
            nc.vector.tensor_scalar(
                out=smf[:, :], in0=new_cs[:, :],
                scalar1=EPS, scalar2=rden[:, :1], op0=alu.add, op1=alu.mult,
            )
            sm = bigp.tile([128, 32], F32)
            nc.vector.tensor_scalar(
                out=sm[:, :], in0=smf[:, :],
                scalar1=n_bc[:, :1], scalar2=None, op0=alu.mult,
            )
            rsm = bigp.tile([128, 32], F32)
            nc.vector.reciprocal(rsm[:, :], sm[:, :])
            new_embed = es  # es fully consumed; reuse the buffer
            nc.vector.tensor_tensor(
                out=new_embed[:, :, :], in0=new_ea[:, :, :],
                in1=rsm[:, :].rearrange("p k -> p k ()").to_broadcast(
                    [128, 32, E_DIM]),
                op=alu.mult,
            )

            # perplexity = exp(-sum(p*log(p+1e-10))), p = counts/65536
            avg = bigp.tile([128, 32], F32)
            nc.vector.tensor_scalar_mul(avg[:, :], counts[:, :], 1.0 / N_TOK)
            avg_e = bigp.tile([128, 32], F32)
            nc.vector.tensor_scalar_add(avg_e[:, :], avg[:, :], 1e-10)
            lg = bigp.tile([128, 32], F32)
            nc.scalar.activation(lg[:, :], avg_e[:, :],
                                 mybir.ActivationFunctionType.Ln)
            pl = bigp.tile([128, 32], F32)
            nc.vector.tensor_tensor(out=pl[:, :], in0=avg[:, :], in1=lg[:, :],
                                    op=alu.mult)
            pls = bigp.tile([128, 1], F32)
            nc.vector.tensor_reduce(out=pls[:, :], in_=pl[:, :],
                                    axis=mybir.AxisListType.X, op=alu.add)
            e_ps = ps_tp.tile([1, 1], F32, tag="comb")
            nc.tensor.matmul(out=e_ps[:, :], lhsT=ones_col[:, :],
                             rhs=pls[:, :], start=True, stop=True)
            ent = bigp.tile([1, 1], F32)
            nc.vector.tensor_scalar_mul(ent[:, :], e_ps[:, :], -1.0)
            ppl = bigp.tile([1, 1], F32)
            nc.scalar.activation(ppl[:, :], ent[:, :],
                                 mybir.ActivationFunctionType.Exp)
            nc.sync.dma_start(out=ppl_out[:, :], in_=ppl[:, :])

            # loss = BETA * loss_total / N
            lt = bigp.tile([1, 1], F32)
            nc.sync.dma_start(out=lt[:, :], in_=stats_red[4096:4097, 0:1])
            lossv = bigp.tile([1, 1], F32)
            nc.vector.tensor_scalar(
                out=lossv[:, :], in0=lt[:, :],
                scalar1=1.0 / (N_TOK * E_DIM), scalar2=BETA,
                op0=alu.mult, op1=alu.mult,
            )
            nc.sync.dma_start(out=loss_out[:, :], in_=lossv[:, :])

            # EMA outputs
            nc.sync.dma_start(
                out=nemb_out[:, :].rearrange("(k p) d -> p k d", p=128),
                in_=new_embed[:, :, :],
            )
            nc.sync.dma_start(
                out=ncs_out[:].rearrange("(k p) -> p k", p=128),
                in_=new_cs[:, :],
            )
            nc.sync.dma_start(
                out=nea_out[:, :].rearrange("(k p) d -> p k d", p=128),
                in_=new_ea[:, :, :],
            )

            # ---------------- straight-through output ----------------
            outT = bigp.tile([E_DIM, T_LOC], F32, tag="zT_slot")
            for i in range(N_TILES):
                tp = ps_tp.tile([E_DIM, 128], F32, tag="tp")
                nc.tensor.transpose(out=tp[:, :], in_=zf[:, i, 0:E_DIM],
                                    identity=ident[:, :])
                nc.scalar.copy(outT[:, i * 128:(i + 1) * 128], tp[:, :])
            nc.sync.dma_start(out=out_sh[:, :], in_=outT[:, :])

    _split_excess_waits(nc)
    return nc


_NC_CACHE = []


def kernel(z, embed_w, cluster_size, embed_avg):
    z = np.ascontiguousarray(z, dtype=np.float32)
    embed_w = np.ascontiguousarray(embed_w, dtype=np.float32)
    cluster_size = np.ascontiguousarray(cluster_size, dtype=np.float32)
    embed_avg = np.ascontiguousarray(embed_avg, dtype=np.float32)

    if not _NC_CACHE:
        _NC_CACHE.append(build_nc())
    nc = _NC_CACHE[0]

    zr = z.reshape(4, 64, 16384)
    in_maps = []
    for c in range(N_CORES):
        b, half = c // 2, c % 2
        zT_c = np.ascontiguousarray(zr[b, :, half * T_LOC:(half + 1) * T_LOC])
        in_maps.append({
            "zT": zT_c,
            "embed_w": embed_w,
            "cluster_size": cluster_size,
            "embed_avg": embed_avg,
        })

    res = run_bass_kernel_spmd(nc, in_maps, core_ids=list(range(N_CORES)))
    rs = res.results

    out = np.empty((4, 64, 16384), dtype=np.float32)
    idx = np.empty((N_CORES, T_LOC), dtype=np.int32)
    for c in range(N_CORES):
        b, half = c // 2, c % 2
        out[b, :, half * T_LOC:(half + 1) * T_LOC] = rs[c]["out_sh"]
        idx[c] = rs[c]["idx_t"].T.reshape(T_LOC)
    out = out.reshape(4, 64, 16, 32, 32)
    idx = idx.reshape(N_TOK)

    r0 = rs[0]
    loss = np.float32(r0["loss"][0, 0])
    ppl = np.float32(r0["ppl"][0, 0])
    return (out, loss, ppl, idx, r0["new_embed"], r0["new_cs"], r0["new_ea"])


# revision 13
# speedup vs baseline: 1.4970x; 1.4426x over previous
"""EMA VectorQuantizer forward pass on 8 TRN2 NeuronCores (Bass/Tile).

Data-parallel over tokens: z [4,64,16,32,32] -> 65536 tokens of dim 64,
8192 tokens per core (channel-major shard [64, 8192] is a natural slice
of z's layout). The [4096,64] codebook is replicated. Per core:
  scores = 2*z@e.T via PE fp32 matmuls (tokens on partitions, codes on
  free dim), top-8 via DVE max8/max_index, then an exact fp32 re-ranking
  of the 8 candidates replicating jax's rounding of
  d = (||z||^2 + ||e||^2) - 2*z@e.T (the (zz+ee) double-rounding decides
  ~3% of tokens' grid-ties, so it must be emulated bit-exactly).
  z_q by indirect row gather; counts/embed_sum by per-tile dedup
  (selection-matrix matmul) + serialized indirect scatter-add DMAs; the
  per-core partial stats are AllReduce'd and the EMA buffer update is
  computed redundantly on every core.
"""

import numpy as np

import bass_rust
import concourse.bass as bass
import concourse.mybir as mybir
import concourse.tile as tile_mod
from concourse.bass import IndirectOffsetOnAxis
from concourse.masks import make_identity
from concourse.tile import TileContext
from concourse.tile_rust import add_dep_helper
from concourse.bass_utils import run_bass_kernel_spmd

N_CORES = 8
N_E = 4096
E_DIM = 64
T_LOC = 8192          # tokens per core
N_TILES = T_LOC // 128
BETA = 0.25
DECAY = 0.99
EPS = 1e-05
N_TOK = 65536
STATS_ROWS = 4224     # 4096 codes + spare rows for scalar partials
F32 = mybir.dt.float32

# ---------------------------------------------------------------------------
# workaround 1: walrus in this container rejects >1 sem wait on the
# TileContext tail drain — pre-absorb the global-clock waits one per drain.

def _patched_drain_and_barrier(self, tick_clock, wait_clock):
    nc = self.nc
    vc = tick_clock.global_clock
    nonzero = [(i, vc[i]) for i in range(len(vc)) if vc[i] > 0]
    for i, t in nonzero:
        pvc = bass_rust.VectorClock([0] * len(vc))
        pvc.require_at_least(i, t)
        nop = nc.sync.drain()
        wait_clock.add_sem_waits(nop.ins, bass_rust.ScopedClock({None: pvc}))
    nc.sync.drain()
    nc.all_engine_barrier()
    assert self.sems is not None
    popped = nc._tile_sem_poison_stack.pop()
    assert popped is self._sem_poison
    nc.clear_and_free_semaphores(list(self.sems.allocated().values()))
    nc.all_engine_barrier()


tile_mod.TileContext._drain_and_barrier = _patched_drain_and_barrier

# ---------------------------------------------------------------------------
# workaround 2: same walrus cap on every other instruction — hoist excess
# semaphore waits onto same-engine NoOps inserted right before it.

_wsplit_ctr = [0]


def _split_excess_waits(nc, max_sem_waits=1):
    for f in nc.m.functions:
        for bb in f.blocks:
            insts = bb.instructions
            new = []
            changed = False
            for inst in insts:
                si = inst.sync_info
                waits = list(si.on_wait) if (si and si.on_wait) else []
                sem_w = [w for w in waits if w.sync_type == "semaphore"]
                other_w = [w for w in waits if w.sync_type != "semaphore"]
                keep = max(0, max_sem_waits - len(other_w))
                if len(sem_w) > keep:
                    excess = sem_w[: len(sem_w) - keep]
                    kept = sem_w[len(sem_w) - keep:]
                    for w in excess:
                        _wsplit_ctr[0] += 1
                        nop = mybir.InstNoOp(
                            name=f"I-wsplit-{_wsplit_ctr[0]}", ins=[], outs=[]
                        )
                        nop.engine = inst.engine
                        nop.sync_info = mybir.SyncInfo(on_wait=[w], on_update=[])
                        new.append(nop)
                    inst.sync_info = mybir.SyncInfo(
                        on_wait=other_w + kept,
                        on_update=list(si.on_update) if si.on_update else [],
                    )
                    changed = True
                new.append(inst)
            if changed:
                insts[:] = new


# ---------------------------------------------------------------------------

def build_nc():
    nc = bass.Bass(trn_type="TRN2", num_devices=N_CORES)
    alu = mybir.AluOpType

    zT_in = nc.dram_tensor("zT", [E_DIM, T_LOC], F32, kind="ExternalInput")
    w_in = nc.dram_tensor("embed_w", [N_E, E_DIM], F32, kind="ExternalInput")
    cs_in = nc.dram_tensor("cluster_size", [N_E], F32, kind="ExternalInput")
    ea_in = nc.dram_tensor("embed_avg", [N_E, E_DIM], F32, kind="ExternalInput")

    out_sh = nc.dram_tensor("out_sh", [E_DIM, T_LOC], F32, kind="ExternalOutput")
    idx_out = nc.dram_tensor("idx_t", [128, N_TILES], mybir.dt.int32,
                             kind="ExternalOutput")
    loss_out = nc.dram_tensor("loss", [1, 1], F32, kind="ExternalOutput")
    ppl_out = nc.dram_tensor("ppl", [1, 1], F32, kind="ExternalOutput")
    nemb_out = nc.dram_tensor("new_embed", [N_E, E_DIM], F32, kind="ExternalOutput")
    ncs_out = nc.dram_tensor("new_cs", [N_E], F32, kind="ExternalOutput")
    nea_out = nc.dram_tensor("new_ea", [N_E, E_DIM], F32, kind="ExternalOutput")

    with TileContext(nc) as tc:
        with (
            tc.tile_pool(name="big", bufs=1) as bigp,      # persistent SBUF
            tc.tile_pool(name="wrk", bufs=2) as wrk,       # per-tile rotating
            tc.tile_pool(name="stg", bufs=2) as stg,       # staged scores
            tc.tile_pool(name="ps_sc", bufs=2, space="PSUM") as ps_sc,
            tc.tile_pool(name="ps_tp", bufs=1, space="PSUM") as ps_tp,
            tc.tile_pool(name="dram", bufs=1, space="DRAM") as dram,
        ):
            # ---------------- constants / setup ----------------
            ident = bigp.tile([128, 128], F32)
            make_identity(nc, ident[:, :])
            iota_row_i = bigp.tile([128, 128], mybir.dt.int32)
            nc.gpsimd.iota(iota_row_i[:, :], pattern=[[1, 128]], base=0,
                           channel_multiplier=0)
            iota_row = bigp.tile([128, 128], F32)
            nc.vector.tensor_copy(iota_row[:, :], iota_row_i[:, :])
            iota_p_i = bigp.tile([128, 1], mybir.dt.int32)
            nc.gpsimd.iota(iota_p_i[:, :], pattern=[[1, 1]], base=0,
                           channel_multiplier=1)
            iota_p = bigp.tile([128, 1], F32)
            nc.vector.tensor_copy(iota_p[:, :], iota_p_i[:, :])
            ltmask = bigp.tile([128, 128], F32)
            nc.vector.tensor_scalar(
                out=ltmask[:, :], in0=iota_row[:, :],
                scalar1=iota_p[:, :1], scalar2=None, op0=alu.is_gt,
            )
            ones_col = bigp.tile([128, 1], F32)
            nc.vector.memset(ones_col[:, :], 1.0)
            ident64 = bigp.tile([64, 64], F32)
            make_identity(nc, ident64[:, :])

            # ---------------- load z (channel-major) and codebook ----------
            zT = bigp.tile([E_DIM, T_LOC], F32, tag="zT_slot")
            nc.sync.dma_start(out=zT[:, :], in_=zT_in[:, :])
            e_sb = bigp.tile([128, 32, E_DIM], F32)
            nc.sync.dma_start(
                out=e_sb[:, :, :],
                in_=w_in[:, :].rearrange("(k p) d -> p k d", p=128),
            )

            # e2T[64, 4096] = 2 * e.T ; ee[4096] = rowwise ||e||^2 (DRAM)
            e2T = bigp.tile([E_DIM, N_E], F32)
            ee_cols = bigp.tile([128, 32], F32)
            scr = bigp.tile([128, E_DIM], F32)
            ee_dram = dram.tile([1, N_E], F32)
            for k in range(32):
                tp = ps_tp.tile([E_DIM, 128], F32, tag="tp")
                nc.tensor.transpose(out=tp[:, :], in_=e_sb[:, k, :],
                                    identity=ident[:, :])
                nc.scalar.mul(e2T[:, k * 128:(k + 1) * 128], tp[:, :], 2.0)
                nc.scalar.activation(
                    scr[:, :], e_sb[:, k, :],
                    mybir.ActivationFunctionType.Square,
                    accum_out=ee_cols[:, k:k + 1],
                )
            nc.sync.dma_start(
                out=ee_dram[0:1, :].rearrange("o (k p) -> (o p) k", p=128),
                in_=ee_cols[:, :],
            )
            negEE = bigp.tile([128, N_E], F32)
            nc.sync.dma_start(
                out=negEE[:, :],
                in_=ee_dram[0:1, :].to_broadcast([128, N_E]),
            )
            nc.vector.tensor_scalar_mul(negEE[:, :], negEE[:, :], -1.0)

            # zf (token-major) + zz per token
            zf = bigp.tile([128, N_TILES, E_DIM + 1], F32)
            nc.vector.memset(zf[:, :, E_DIM:E_DIM + 1], 1.0)
            zz_all = bigp.tile([128, N_TILES], F32)
            for i in range(N_TILES):
                tp = ps_tp.tile([128, E_DIM], F32, tag="tp")
                nc.tensor.transpose(
                    out=tp[:, :], in_=zT[:, i * 128:(i + 1) * 128],
                    identity=ident64[:, :],
                )
                nc.vector.tensor_copy(zf[:, i, 0:E_DIM], tp[:, :])
                nc.scalar.activation(
                    scr[:, :], zf[:, i, 0:E_DIM],
                    mybir.ActivationFunctionType.Square,
                    accum_out=zz_all[:, i:i + 1],
                )

            # stats accumulator in DRAM (zeroed): rows 0..4095 = [embed_sum |
            # count]; row 4096 col 0 = loss partial
            negzz_all = bigp.tile([128, N_TILES], F32)
            nc.vector.tensor_scalar_mul(negzz_all[:, :], zz_all[:, :], -1.0)
            N_ROT = 8
            stats = dram.tile([STATS_ROWS, 65], F32)
            stats_bufs = []
            for r in range(N_ROT):
                srot = dram.tile([STATS_ROWS, 65], F32, tag=f"srot{r}")
                stats_bufs.append(srot)
            zb = bigp.tile([128, STATS_ROWS * 65 // 128], F32)
            nc.vector.memset(zb[:, :], 0.0)
            znop = nc.gpsimd.engine_nop()
            for sb in stats_bufs:
                zd = nc.sync.dma_start(
                    out=sb[:, :].rearrange("a b -> (a b)")
                                .rearrange("(p k) -> p k", p=128),
                    in_=zb[:, :],
                )
                add_dep_helper(znop.ins, zd.ins, sync=True,
                               reason="gpsimd observes stats zeroing")

            bc_reg = nc.gpsimd.to_reg(N_E - 1)
            idx_all = bigp.tile([128, N_TILES], mybir.dt.int32)
            idx_allf = bigp.tile([128, N_TILES], F32)
            lcols = bigp.tile([128, N_TILES], F32)
            comb_all = bigp.tile([128, N_TILES, 65], F32)
            ieff_all = bigp.tile([128, N_TILES], mybir.dt.int32)

            # ---------------- main loop over 64 token tiles ----------------
            for i in range(N_TILES):
                lhsT = zT[:, i * 128:(i + 1) * 128]
                staged = stg.tile([128, N_E + 8], F32, tag="staged")
                nc.vector.memset(staged[:, N_E:N_E + 8], -1e30)
                for base, width in ((0, 1536), (1536, 1536), (3072, 1024)):
                    ps = ps_sc.tile([128, width], F32, tag="sc")
                    for c in range(width // 512):
                        col = base + c * 512
                        nc.tensor.matmul(
                            out=ps[:, c * 512:(c + 1) * 512],
                            lhsT=lhsT,
                            rhs=e2T[:, col:col + 512],
                            start=True, stop=True,
                        )
                    # staged = fl(fl(-ee - zz) + 2*z@e) = -d with jax's
                    # exact double rounding; max over it = argmin of d
                    nc.vector.scalar_tensor_tensor(
                        out=staged[:, base:base + width],
                        in0=negEE[:, base:base + width],
                        scalar=negzz_all[:, i:i + 1],
                        in1=ps[:, 0:width],
                        op0=alu.add, op1=alu.add,
                    )

                v8 = wrk.tile([128, 8], F32, tag="v8")
                nc.vector.max(out=v8[:, :], in_=staged[:, 0:N_E + 8])
                i8 = wrk.tile([128, 8], mybir.dt.uint32, tag="i8")
                nc.vector.max_index(out=i8[:, :], in_max=v8[:, :],
                                    in_values=staged[:, 0:N_E + 8])
                i8f = wrk.tile([128, 8], F32, tag="i8f")
                nc.vector.tensor_copy(i8f[:, 0:1], i8[:, 0:1])
                nc.vector.tensor_copy(idx_allf[:, i:i + 1], i8f[:, 0:1])
                nc.vector.tensor_copy(idx_all[:, i:i + 1], i8f[:, 0:1])

                # z_q gather, straight-through out, loss contribution
                zq = wrk.tile([128, E_DIM], F32, tag="zq")
                nc.gpsimd.indirect_dma_start(
                    out=zq[:, :], out_offset=None,
                    in_=w_in[:, :],
                    in_offset=IndirectOffsetOnAxis(ap=idx_all[:, i:i + 1], axis=0),
                )
                dlt = wrk.tile([128, E_DIM], F32, tag="dlt")
                nc.vector.tensor_tensor(out=dlt[:, :], in0=zq[:, :],
                                        in1=zf[:, i, 0:E_DIM], op=alu.subtract)
                nc.scalar.activation(
                    scr[:, :], dlt[:, :],
                    mybir.ActivationFunctionType.Square,
                    accum_out=lcols[:, i:i + 1],
                )

                # in-tile dedup for the scatter: S = (idx == idx.T)
                tp = ps_tp.tile([128, 128], F32, tag="tp")
                nc.tensor.transpose(
                    out=tp[:, :],
                    in_=idx_allf[:, i:i + 1].to_broadcast([128, 128]),
                    identity=ident[:, :],
                )
                idxT = wrk.tile([128, 128], F32, tag="idxT")
                nc.vector.tensor_copy(idxT[:, :], tp[:, :])
                S = wrk.tile([128, 128], F32, tag="S")
                nc.vector.tensor_tensor(
                    out=S[:, :],
                    in0=idx_allf[:, i:i + 1].to_broadcast([128, 128]),
                    in1=idxT[:, :], op=alu.is_equal,
                )
                comb_ps = ps_tp.tile([128, 65], F32, tag="comb")
                nc.tensor.matmul(out=comb_ps[:, :], lhsT=S[:, :],
                                 rhs=zf[:, i, :], start=True, stop=True)
                nc.vector.tensor_copy(comb_all[:, i, :], comb_ps[:, :])
                SL = wrk.tile([128, 128], F32, tag="SL")
                nc.vector.tensor_tensor(out=SL[:, :], in0=S[:, :],
                                        in1=ltmask[:, :], op=alu.mult)
                cnt = wrk.tile([128, 1], F32, tag="cnt")
                nc.vector.tensor_reduce(out=cnt[:, :], in_=SL[:, :],
                                        axis=mybir.AxisListType.X, op=alu.add)
                notfirst = wrk.tile([128, 1], F32, tag="nf")
                nc.vector.tensor_scalar(
                    out=notfirst[:, :], in0=cnt[:, :],
                    scalar1=0.0, scalar2=None, op0=alu.is_gt,
                )
                idx_eff_f = wrk.tile([128, 1], F32, tag="ieff_f")
                nc.vector.scalar_tensor_tensor(
                    out=idx_eff_f[:, :], in0=notfirst[:, :], scalar=60000.0,
                    in1=idx_allf[:, i:i + 1], op0=alu.mult, op1=alu.add,
                )
                nc.vector.tensor_copy(ieff_all[:, i:i + 1], idx_eff_f[:, :])
                sc_dma = nc.gpsimd.indirect_dma_start(
                    out=stats_bufs[i % N_ROT][:, :],
                    out_offset=IndirectOffsetOnAxis(ap=ieff_all[:, i:i + 1], axis=0),
                    in_=comb_all[:, i, :], in_offset=None,
                    compute_op=alu.add,
                    bounds_check=bc_reg, oob_is_err=False,
                )
                add_dep_helper(sc_dma.ins, znop.ins, sync=False,
                               reason="scatter after stats zeroing")

                # straight-through output overwrites zf slot:
                # out = zp + (z_q - zp), computed exactly like jax
                nc.vector.tensor_tensor(out=zf[:, i, 0:E_DIM],
                                        in0=zf[:, i, 0:E_DIM],
                                        in1=dlt[:, :], op=alu.add)

            # idx output
            nc.sync.dma_start(out=idx_out[:, :], in_=idx_all[:, :])

            # ---------------- loss partial -> stats row 4096 ---------------
            lsum = bigp.tile([128, 1], F32)
            nc.vector.tensor_reduce(out=lsum[:, :], in_=lcols[:, :],
                                    axis=mybir.AxisListType.X, op=alu.add)
            lt_ps = ps_tp.tile([1, 1], F32, tag="comb")
            nc.tensor.matmul(out=lt_ps[:, :], lhsT=ones_col[:, :],
                             rhs=lsum[:, :], start=True, stop=True)
            lpart = bigp.tile([1, 1], F32)
            nc.vector.tensor_copy(lpart[:, :], lt_ps[:, :])
            lp_dma = nc.sync.dma_start(out=stats_bufs[0][4096:4097, 0:1],
                                       in_=lpart[:, :])
            add_dep_helper(lp_dma.ins, znop.ins, sync=False,
                           reason="partial write after zeroing")
            # merge the rotated partials into the collective input
            FLAT = STATS_ROWS * 65 // 128
            macc = bigp.tile([128, FLAT], F32)
            mtmp = zb  # zeroing source is dead by now; reuse as merge temp
            nc.sync.dma_start(
                out=macc[:, :],
                in_=stats_bufs[0][:, :].rearrange("a b -> (a b)")
                                       .rearrange("(p k) -> p k", p=128))
            for r in range(1, N_ROT):
                nc.sync.dma_start(
                    out=mtmp[:, :],
                    in_=stats_bufs[r][:, :].rearrange("a b -> (a b)")
                                           .rearrange("(p k) -> p k", p=128))
                nc.vector.tensor_tensor(out=macc[:, :], in0=macc[:, :],
                                        in1=mtmp[:, :], op=alu.add)
            nc.sync.dma_start(
                out=stats[:, :].rearrange("a b -> (a b)")
                               .rearrange("(p k) -> p k", p=128),
                in_=macc[:, :])

            # ---------------- all-reduce the stats ----------------
            stats_red = dram.tile([STATS_ROWS, 65], F32)
            cc = nc.gpsimd.collective_compute(
                "AllReduce", alu.add,
                replica_groups=[list(range(N_CORES))],
                ins=[stats[:, :]], outs=[stats_red[:, :]],
            )

            # ---------------- EMA update + scalars (redundant per core) ----
            es = bigp.tile([128, 32, E_DIM], F32)
            nc.sync.dma_start(
                out=es[:, :, :],
                in_=stats_red[0:N_E, 0:E_DIM].rearrange(
                    "(k p) d -> p k d", p=128),
            )
            counts = bigp.tile([128, 32], F32)
            nc.sync.dma_start(
                out=counts[:, :],
                in_=stats_red[0:N_E, E_DIM:E_DIM + 1].rearrange(
                    "(k p) o -> p (k o)", p=128),
            )
            cs_sb = bigp.tile([128, 32], F32)
            nc.sync.dma_start(
                out=cs_sb[:, :],
                in_=cs_in[:].rearrange("(k p) -> p k", p=128),
            )
            ea_sb = bigp.tile([128, 32, E_DIM], F32)
            nc.sync.dma_start(
                out=ea_sb[:, :, :],
                in_=ea_in[:, :].rearrange("(k p) d -> p k d", p=128),
            )

            # new_cs = fl(cs*0.99) + fl(counts*0.01)
            cs99 = bigp.tile([128, 32], F32)
            nc.vector.tensor_scalar_mul(cs99[:, :], cs_sb[:, :], DECAY)
            new_cs = bigp.tile([128, 32], F32)
            nc.vector.scalar_tensor_tensor(
                out=new_cs[:, :], in0=counts[:, :], scalar=1.0 - DECAY,
                in1=cs99[:, :], op0=alu.mult, op1=alu.add,
            )
            # new_ea = fl(ea*0.99) + fl(es*0.01)  (overwrite ea_sb)
            nc.vector.tensor_scalar_mul(ea_sb[:, :, :], ea_sb[:, :, :], DECAY)
            new_ea = bigp.tile([128, 32, E_DIM], F32)
            nc.vector.scalar_tensor_tensor(
                out=new_ea[:, :, :], in0=es[:, :, :], scalar=1.0 - DECAY,
                in1=ea_sb[:, :, :], op0=alu.mult, op1=alu.add,
            )

            # n = sum(new_cs) -> broadcast to all partitions via PE
            ncs_sum = bigp.tile([1, 32], F32)
            s_ps = ps_tp.tile([1, 32], F32, tag="comb")
            nc.tensor.matmul(out=s_ps[:, :], lhsT=ones_col[:, :],
                             rhs=new_cs[:, :], start=True, stop=True)
            nc.vector.tensor_copy(ncs_sum[:, :], s_ps[:, :])
            n_11 = bigp.tile([1, 1], F32)
            nc.vector.tensor_reduce(out=n_11[:, :], in_=ncs_sum[:, :],
                                    axis=mybir.AxisListType.X, op=alu.add)
            nb_ps = ps_tp.tile([128, 1], F32, tag="comb")
            nc.tensor.matmul(out=nb_ps[:, :],
                             lhsT=ones_col[0:1, :].to_broadcast([1, 128]),
                             rhs=n_11[:, :], start=True, stop=True)
            n_bc = bigp.tile([128, 1], F32)
            nc.vector.tensor_copy(n_bc[:, :], nb_ps[:, :])

            # smoothed = (new_cs + eps) / (n + N_E*eps) * n
            denom = bigp.tile([128, 1], F32)
            nc.vector.tensor_scalar_add(denom[:, :], n_bc[:, :], N_E * EPS)
            rden = bigp.tile([128, 1], F32)
            nc.vector.reciprocal(rden[:, :], denom[:, :])
            smf = bigp.tile([128, 32], F32)
            nc.vector.tensor_scalar(
                out=smf[:, :], in0=new_cs[:, :],
                scalar1=EPS, scalar2=rden[:, :1], op0=alu.add, op1=alu.mult,
            )
            sm = bigp.tile([128, 32], F32)
            nc.vector.tensor_scalar(
                out=sm[:, :], in0=smf[:, :],
                scalar1=n_bc[:, :1], scalar2=None, op0=alu.mult,
            )
            rsm = bigp.tile([128, 32], F32)
            nc.vector.reciprocal(rsm[:, :], sm[:, :])
            new_embed = es  # es fully consumed; reuse the buffer
            nc.vector.tensor_tensor(
                out=new_embed[:, :, :], in0=new_ea[:, :, :],
                in1=rsm[:, :].rearrange("p k -> p k ()").to_broadcast(
                    [128, 32, E_DIM]),
                op=alu.mult,
            )

            # perplexity = exp(-sum(p*log(p+1e-10))), p = counts/65536
            avg = bigp.tile([128, 32], F32)
            nc.vector.tensor_scalar_mul(avg[:, :], counts[:, :], 1.0 / N_TOK)
            avg_e = bigp.tile([128, 32], F32)
            nc.vector.tensor_scalar_add(avg_e[:, :], avg[:, :], 1e-10)
            lg = bigp.tile([128, 32], F32)
            nc.scalar.activation(lg[:, :], avg_e[:, :],
                                 mybir.ActivationFunctionType.Ln)
            pl = bigp.tile([128, 32], F32)
            nc.vector.tensor_tensor(out=pl[:, :], in0=avg[:, :], in1=lg[:, :],
                                    op=alu.mult)
            pls = bigp.tile([128, 1], F32)
            nc.vector.tensor_reduce(out=pls[:, :], in_=pl[:, :],
                                    axis=mybir.AxisListType.X, op=alu.add)
            e_ps = ps_tp.tile([1, 1], F32, tag="comb")
            nc.tensor.matmul(out=e_ps[:, :], lhsT=ones_col[:, :],
                             rhs=pls[:, :], start=True, stop=True)
            ent = bigp.tile([1, 1], F32)
            nc.vector.tensor_scalar_mul(ent[:, :], e_ps[:, :], -1.0)
            ppl = bigp.tile([1, 1], F32)
            nc.scalar.activation(ppl[:, :], ent[:, :],
                                 mybir.ActivationFunctionType.Exp)
            nc.sync.dma_start(out=ppl_out[:, :], in_=ppl[:, :])

            # loss = BETA * loss_total / N
            lt = bigp.tile([1, 1], F32)
            nc.sync.dma_start(out=lt[:, :], in_=stats_red[4096:4097, 0:1])
            lossv = bigp.tile([1, 1], F32)
            nc.vector.tensor_scalar(
                out=lossv[:, :], in0=lt[:, :],
                scalar1=1.0 / (N_TOK * E_DIM), scalar2=BETA,
                op0=alu.mult, op1=alu.mult,
            )
            nc.sync.dma_start(out=loss_out[:, :], in_=lossv[:, :])

            # EMA outputs
            nc.sync.dma_start(
                out=nemb_out[:, :].rearrange("(k p) d -> p k d", p=128),
                in_=new_embed[:, :, :],
            )
            nc.sync.dma_start(
                out=ncs_out[:].rearrange("(k p) -> p k", p=128),
                in_=new_cs[:, :],
            )
            nc.sync.dma_start(
                out=nea_out[:, :].rearrange("(k p) d -> p k d", p=128),
                in_=new_ea[:, :, :],
            )

            # ---------------- straight-through output ----------------
            outT = bigp.tile([E_DIM, T_LOC], F32, tag="zT_slot")
            for i in range(N_TILES):
                tp = ps_tp.tile([E_DIM, 128], F32, tag="tp")
                nc.tensor.transpose(out=tp[:, :], in_=zf[:, i, 0:E_DIM],
                                    identity=ident[:, :])
                nc.scalar.copy(outT[:, i * 128:(i + 1) * 128], tp[:, :])
            nc.sync.dma_start(out=out_sh[:, :], in_=outT[:, :])

    _split_excess_waits(nc)
    return nc


_NC_CACHE = []


def kernel(z, embed_w, cluster_size, embed_avg):
    z = np.ascontiguousarray(z, dtype=np.float32)
    embed_w = np.ascontiguousarray(embed_w, dtype=np.float32)
    cluster_size = np.ascontiguousarray(cluster_size, dtype=np.float32)
    embed_avg = np.ascontiguousarray(embed_avg, dtype=np.float32)

    if not _NC_CACHE:
        _NC_CACHE.append(build_nc())
    nc = _NC_CACHE[0]

    zr = z.reshape(4, 64, 16384)
    in_maps = []
    for c in range(N_CORES):
        b, half = c // 2, c % 2
        zT_c = np.ascontiguousarray(zr[b, :, half * T_LOC:(half + 1) * T_LOC])
        in_maps.append({
            "zT": zT_c,
            "embed_w": embed_w,
            "cluster_size": cluster_size,
            "embed_avg": embed_avg,
        })

    res = run_bass_kernel_spmd(nc, in_maps, core_ids=list(range(N_CORES)))
    rs = res.results

    out = np.empty((4, 64, 16384), dtype=np.float32)
    idx = np.empty((N_CORES, T_LOC), dtype=np.int32)
    for c in range(N_CORES):
        b, half = c // 2, c % 2
        out[b, :, half * T_LOC:(half + 1) * T_LOC] = rs[c]["out_sh"]
        idx[c] = rs[c]["idx_t"].T.reshape(T_LOC)
    out = out.reshape(4, 64, 16, 32, 32)
    idx = idx.reshape(N_TOK)

    r0 = rs[0]
    loss = np.float32(r0["loss"][0, 0])
    ppl = np.float32(r0["ppl"][0, 0])
    return (out, loss, ppl, idx, r0["new_embed"], r0["new_cs"], r0["new_ea"])


# revision 14
# speedup vs baseline: 1.5580x; 1.0408x over previous
"""EMA VectorQuantizer forward pass on 8 TRN2 NeuronCores (Bass/Tile).

Data-parallel over tokens: z [4,64,16,32,32] -> 65536 tokens of dim 64,
8192 tokens per core (channel-major shard [64, 8192] is a natural slice
of z's layout). The [4096,64] codebook is replicated. Per core:
  scores = 2*z@e.T via PE fp32 matmuls (tokens on partitions, codes on
  free dim), top-8 via DVE max8/max_index, then an exact fp32 re-ranking
  of the 8 candidates replicating jax's rounding of
  d = (||z||^2 + ||e||^2) - 2*z@e.T (the (zz+ee) double-rounding decides
  ~3% of tokens' grid-ties, so it must be emulated bit-exactly).
  z_q by indirect row gather; counts/embed_sum by per-tile dedup
  (selection-matrix matmul) + serialized indirect scatter-add DMAs; the
  per-core partial stats are AllReduce'd and the EMA buffer update is
  computed redundantly on every core.
"""

import numpy as np

import bass_rust
import concourse.bass as bass
import concourse.mybir as mybir
import concourse.tile as tile_mod
from concourse.bass import IndirectOffsetOnAxis
from concourse.masks import make_identity
from concourse.tile import TileContext
from concourse.tile_rust import add_dep_helper
from concourse.bass_utils import run_bass_kernel_spmd

N_CORES = 8
N_E = 4096
E_DIM = 64
T_LOC = 8192          # tokens per core
N_TILES = T_LOC // 128
BETA = 0.25
DECAY = 0.99
EPS = 1e-05
N_TOK = 65536
STATS_ROWS = 4096
N_SL = N_E // N_CORES     # per-core code slice for the EMA update
K_SL = N_SL // 128        # k-tiles per slice
F32 = mybir.dt.float32

# ---------------------------------------------------------------------------
# workaround 1: walrus in this container rejects >1 sem wait on the
# TileContext tail drain — pre-absorb the global-clock waits one per drain.

def _patched_drain_and_barrier(self, tick_clock, wait_clock):
    nc = self.nc
    vc = tick_clock.global_clock
    nonzero = [(i, vc[i]) for i in range(len(vc)) if vc[i] > 0]
    for i, t in nonzero:
        pvc = bass_rust.VectorClock([0] * len(vc))
        pvc.require_at_least(i, t)
        nop = nc.sync.drain()
        wait_clock.add_sem_waits(nop.ins, bass_rust.ScopedClock({None: pvc}))
    nc.sync.drain()
    nc.all_engine_barrier()
    assert self.sems is not None
    popped = nc._tile_sem_poison_stack.pop()
    assert popped is self._sem_poison
    nc.clear_and_free_semaphores(list(self.sems.allocated().values()))
    nc.all_engine_barrier()


tile_mod.TileContext._drain_and_barrier = _patched_drain_and_barrier

# ---------------------------------------------------------------------------
# workaround 2: same walrus cap on every other instruction — hoist excess
# semaphore waits onto same-engine NoOps inserted right before it.

_wsplit_ctr = [0]


def _split_excess_waits(nc, max_sem_waits=1):
    for f in nc.m.functions:
        for bb in f.blocks:
            insts = bb.instructions
            new = []
            changed = False
            for inst in insts:
                si = inst.sync_info
                waits = list(si.on_wait) if (si and si.on_wait) else []
                sem_w = [w for w in waits if w.sync_type == "semaphore"]
                other_w = [w for w in waits if w.sync_type != "semaphore"]
                keep = max(0, max_sem_waits - len(other_w))
                if len(sem_w) > keep:
                    excess = sem_w[: len(sem_w) - keep]
                    kept = sem_w[len(sem_w) - keep:]
                    for w in excess:
                        _wsplit_ctr[0] += 1
                        nop = mybir.InstNoOp(
                            name=f"I-wsplit-{_wsplit_ctr[0]}", ins=[], outs=[]
                        )
                        nop.engine = inst.engine
                        nop.sync_info = mybir.SyncInfo(on_wait=[w], on_update=[])
                        new.append(nop)
                    inst.sync_info = mybir.SyncInfo(
                        on_wait=other_w + kept,
                        on_update=list(si.on_update) if si.on_update else [],
                    )
                    changed = True
                new.append(inst)
            if changed:
                insts[:] = new


# ---------------------------------------------------------------------------

def build_nc():
    nc = bass.Bass(trn_type="TRN2", num_devices=N_CORES)
    alu = mybir.AluOpType

    zT_in = nc.dram_tensor("zT", [E_DIM, T_LOC], F32, kind="ExternalInput")
    w_in = nc.dram_tensor("embed_w", [N_E, E_DIM], F32, kind="ExternalInput")
    cs_in = nc.dram_tensor("cluster_size", [N_SL], F32, kind="ExternalInput")
    ea_in = nc.dram_tensor("embed_avg", [N_SL, E_DIM], F32, kind="ExternalInput")

    out_sh = nc.dram_tensor("out_sh", [E_DIM, T_LOC], F32, kind="ExternalOutput")
    idx_out = nc.dram_tensor("idx_t", [128, N_TILES], mybir.dt.int32,
                             kind="ExternalOutput")
    loss_out = nc.dram_tensor("loss", [1, 1], F32, kind="ExternalOutput")
    ppl_out = nc.dram_tensor("ppl", [1, 1], F32, kind="ExternalOutput")
    nemb_out = nc.dram_tensor("new_embed", [N_SL, E_DIM], F32, kind="ExternalOutput")
    ncs_out = nc.dram_tensor("new_cs", [N_SL], F32, kind="ExternalOutput")
    nea_out = nc.dram_tensor("new_ea", [N_SL, E_DIM], F32, kind="ExternalOutput")

    with TileContext(nc) as tc:
        with (
            tc.tile_pool(name="big", bufs=1) as bigp,      # persistent SBUF
            tc.tile_pool(name="wrk", bufs=2) as wrk,       # per-tile rotating
            tc.tile_pool(name="stg", bufs=2) as stg,       # staged scores
            tc.tile_pool(name="ps_sc", bufs=2, space="PSUM") as ps_sc,
            tc.tile_pool(name="ps_tp", bufs=1, space="PSUM") as ps_tp,
            tc.tile_pool(name="dram", bufs=1, space="DRAM") as dram,
        ):
            # ---------------- constants / setup ----------------
            ident = bigp.tile([128, 128], F32)
            make_identity(nc, ident[:, :])
            iota_row_i = bigp.tile([128, 128], mybir.dt.int32)
            nc.gpsimd.iota(iota_row_i[:, :], pattern=[[1, 128]], base=0,
                           channel_multiplier=0)
            iota_row = bigp.tile([128, 128], F32)
            nc.vector.tensor_copy(iota_row[:, :], iota_row_i[:, :])
            iota_p_i = bigp.tile([128, 1], mybir.dt.int32)
            nc.gpsimd.iota(iota_p_i[:, :], pattern=[[1, 1]], base=0,
                           channel_multiplier=1)
            iota_p = bigp.tile([128, 1], F32)
            nc.vector.tensor_copy(iota_p[:, :], iota_p_i[:, :])
            ltmask = bigp.tile([128, 128], F32)
            nc.vector.tensor_scalar(
                out=ltmask[:, :], in0=iota_row[:, :],
                scalar1=iota_p[:, :1], scalar2=None, op0=alu.is_gt,
            )
            ones_col = bigp.tile([128, 1], F32)
            nc.vector.memset(ones_col[:, :], 1.0)
            ident64 = bigp.tile([64, 64], F32)
            make_identity(nc, ident64[:, :])

            # ---------------- load z (channel-major) and codebook ----------
            zT = bigp.tile([E_DIM, T_LOC], F32, tag="zT_slot")
            nc.sync.dma_start(out=zT[:, :], in_=zT_in[:, :])
            e_sb = bigp.tile([128, 32, E_DIM], F32)
            nc.sync.dma_start(
                out=e_sb[:, :, :],
                in_=w_in[:, :].rearrange("(k p) d -> p k d", p=128),
            )

            # e2T[64, 4096] = 2 * e.T ; ee[4096] = rowwise ||e||^2 (DRAM)
            e2T = bigp.tile([E_DIM, N_E], F32)
            ee_cols = bigp.tile([128, 32], F32)
            scr = bigp.tile([128, E_DIM], F32)
            ee_dram = dram.tile([1, N_E], F32)
            for k in range(32):
                tp = ps_tp.tile([E_DIM, 128], F32, tag="tp")
                nc.tensor.transpose(out=tp[:, :], in_=e_sb[:, k, :],
                                    identity=ident[:, :])
                nc.scalar.mul(e2T[:, k * 128:(k + 1) * 128], tp[:, :], 2.0)
                nc.scalar.activation(
                    scr[:, :], e_sb[:, k, :],
                    mybir.ActivationFunctionType.Square,
                    accum_out=ee_cols[:, k:k + 1],
                )
            nc.sync.dma_start(
                out=ee_dram[0:1, :].rearrange("o (k p) -> (o p) k", p=128),
                in_=ee_cols[:, :],
            )
            negEE = bigp.tile([128, N_E], F32)
            nc.sync.dma_start(
                out=negEE[:, :],
                in_=ee_dram[0:1, :].to_broadcast([128, N_E]),
            )
            nc.vector.tensor_scalar_mul(negEE[:, :], negEE[:, :], -1.0)

            # zf (token-major) + zz per token
            zf = bigp.tile([128, N_TILES, E_DIM + 1], F32)
            nc.vector.memset(zf[:, :, E_DIM:E_DIM + 1], 1.0)
            zz_all = bigp.tile([128, N_TILES], F32)
            for i in range(N_TILES):
                tp = ps_tp.tile([128, E_DIM], F32, tag="tp")
                nc.tensor.transpose(
                    out=tp[:, :], in_=zT[:, i * 128:(i + 1) * 128],
                    identity=ident64[:, :],
                )
                nc.vector.tensor_copy(zf[:, i, 0:E_DIM], tp[:, :])
                nc.scalar.activation(
                    scr[:, :], zf[:, i, 0:E_DIM],
                    mybir.ActivationFunctionType.Square,
                    accum_out=zz_all[:, i:i + 1],
                )

            # stats accumulator in DRAM (zeroed): rows 0..4095 = [embed_sum |
            # count]; row 4096 col 0 = loss partial
            negzz_all = bigp.tile([128, N_TILES], F32)
            nc.vector.tensor_scalar_mul(negzz_all[:, :], zz_all[:, :], -1.0)
            N_ROT = 8
            stats = dram.tile([STATS_ROWS, 65], F32)
            stats_bufs = []
            for r in range(N_ROT):
                srot = dram.tile([STATS_ROWS, 65], F32, tag=f"srot{r}")
                stats_bufs.append(srot)
            zb = bigp.tile([128, STATS_ROWS * 65 // 128], F32)
            nc.vector.memset(zb[:, :], 0.0)
            znop = nc.gpsimd.engine_nop()
            for sb in stats_bufs:
                zd = nc.sync.dma_start(
                    out=sb[:, :].rearrange("a b -> (a b)")
                                .rearrange("(p k) -> p k", p=128),
                    in_=zb[:, :],
                )
                add_dep_helper(znop.ins, zd.ins, sync=True,
                               reason="gpsimd observes stats zeroing")

            bc_reg = nc.gpsimd.to_reg(N_E - 1)
            idx_all = bigp.tile([128, N_TILES], mybir.dt.int32)
            idx_allf = bigp.tile([128, N_TILES], F32)
            lcols = bigp.tile([128, N_TILES], F32)
            comb_all = bigp.tile([128, N_TILES, 65], F32)
            ieff_all = bigp.tile([128, N_TILES], mybir.dt.int32)

            # ---------------- main loop over 64 token tiles ----------------
            for i in range(N_TILES):
                lhsT = zT[:, i * 128:(i + 1) * 128]
                staged = stg.tile([128, N_E + 8], F32, tag="staged")
                nc.vector.memset(staged[:, N_E:N_E + 8], -1e30)
                for base, width in ((0, 1536), (1536, 1536), (3072, 1024)):
                    ps = ps_sc.tile([128, width], F32, tag="sc")
                    for c in range(width // 512):
                        col = base + c * 512
                        nc.tensor.matmul(
                            out=ps[:, c * 512:(c + 1) * 512],
                            lhsT=lhsT,
                            rhs=e2T[:, col:col + 512],
                            start=True, stop=True,
                        )
                    # staged = fl(fl(-ee - zz) + 2*z@e) = -d with jax's
                    # exact double rounding; max over it = argmin of d
                    nc.vector.scalar_tensor_tensor(
                        out=staged[:, base:base + width],
                        in0=negEE[:, base:base + width],
                        scalar=negzz_all[:, i:i + 1],
                        in1=ps[:, 0:width],
                        op0=alu.add, op1=alu.add,
                    )

                v8 = wrk.tile([128, 8], F32, tag="v8")
                nc.vector.max(out=v8[:, :], in_=staged[:, 0:N_E + 8])
                i8 = wrk.tile([128, 8], mybir.dt.uint32, tag="i8")
                nc.vector.max_index(out=i8[:, :], in_max=v8[:, :],
                                    in_values=staged[:, 0:N_E + 8])
                i8f = wrk.tile([128, 8], F32, tag="i8f")
                nc.vector.tensor_copy(i8f[:, 0:1], i8[:, 0:1])
                nc.vector.tensor_copy(idx_allf[:, i:i + 1], i8f[:, 0:1])
                nc.vector.tensor_copy(idx_all[:, i:i + 1], i8f[:, 0:1])

                # z_q gather, straight-through out, loss contribution
                zq = wrk.tile([128, E_DIM], F32, tag="zq")
                nc.gpsimd.indirect_dma_start(
                    out=zq[:, :], out_offset=None,
                    in_=w_in[:, :],
                    in_offset=IndirectOffsetOnAxis(ap=idx_all[:, i:i + 1], axis=0),
                )
                dlt = wrk.tile([128, E_DIM], F32, tag="dlt")
                nc.vector.tensor_tensor(out=dlt[:, :], in0=zq[:, :],
                                        in1=zf[:, i, 0:E_DIM], op=alu.subtract)
                nc.scalar.activation(
                    scr[:, :], dlt[:, :],
                    mybir.ActivationFunctionType.Square,
                    accum_out=lcols[:, i:i + 1],
                )

                # in-tile dedup for the scatter: S = (idx == idx.T)
                tp = ps_tp.tile([128, 128], F32, tag="tp")
                nc.tensor.transpose(
                    out=tp[:, :],
                    in_=idx_allf[:, i:i + 1].to_broadcast([128, 128]),
                    identity=ident[:, :],
                )
                idxT = wrk.tile([128, 128], F32, tag="idxT")
                nc.vector.tensor_copy(idxT[:, :], tp[:, :])
                S = wrk.tile([128, 128], F32, tag="S")
                nc.vector.tensor_tensor(
                    out=S[:, :],
                    in0=idx_allf[:, i:i + 1].to_broadcast([128, 128]),
                    in1=idxT[:, :], op=alu.is_equal,
                )
                comb_ps = ps_tp.tile([128, 65], F32, tag="comb")
                nc.tensor.matmul(out=comb_ps[:, :], lhsT=S[:, :],
                                 rhs=zf[:, i, :], start=True, stop=True)
                nc.vector.tensor_copy(comb_all[:, i, :], comb_ps[:, :])
                SL = wrk.tile([128, 128], F32, tag="SL")
                nc.vector.tensor_tensor(out=SL[:, :], in0=S[:, :],
                                        in1=ltmask[:, :], op=alu.mult)
                cnt = wrk.tile([128, 1], F32, tag="cnt")
                nc.vector.tensor_reduce(out=cnt[:, :], in_=SL[:, :],
                                        axis=mybir.AxisListType.X, op=alu.add)
                notfirst = wrk.tile([128, 1], F32, tag="nf")
                nc.vector.tensor_scalar(
                    out=notfirst[:, :], in0=cnt[:, :],
                    scalar1=0.0, scalar2=None, op0=alu.is_gt,
                )
                idx_eff_f = wrk.tile([128, 1], F32, tag="ieff_f")
                nc.vector.scalar_tensor_tensor(
                    out=idx_eff_f[:, :], in0=notfirst[:, :], scalar=60000.0,
                    in1=idx_allf[:, i:i + 1], op0=alu.mult, op1=alu.add,
                )
                nc.vector.tensor_copy(ieff_all[:, i:i + 1], idx_eff_f[:, :])
                sc_dma = nc.gpsimd.indirect_dma_start(
                    out=stats_bufs[i % N_ROT][:, :],
                    out_offset=IndirectOffsetOnAxis(ap=ieff_all[:, i:i + 1], axis=0),
                    in_=comb_all[:, i, :], in_offset=None,
                    compute_op=alu.add,
                    bounds_check=bc_reg, oob_is_err=False,
                )
                add_dep_helper(sc_dma.ins, znop.ins, sync=False,
                               reason="scatter after stats zeroing")

                # straight-through output overwrites zf slot:
                # out = zp + (z_q - zp), computed exactly like jax
                nc.vector.tensor_tensor(out=zf[:, i, 0:E_DIM],
                                        in0=zf[:, i, 0:E_DIM],
                                        in1=dlt[:, :], op=alu.add)

            # idx output
            nc.sync.dma_start(out=idx_out[:, :], in_=idx_all[:, :])

            # ---------------- loss partial ----------------
            lsum = bigp.tile([128, 1], F32)
            nc.vector.tensor_reduce(out=lsum[:, :], in_=lcols[:, :],
                                    axis=mybir.AxisListType.X, op=alu.add)
            lt_ps = ps_tp.tile([1, 1], F32, tag="comb")
            nc.tensor.matmul(out=lt_ps[:, :], lhsT=ones_col[:, :],
                             rhs=lsum[:, :], start=True, stop=True)
            lpart = bigp.tile([1, 1], F32)
            nc.vector.tensor_copy(lpart[:, :], lt_ps[:, :])

            # merge the rotated partials into the collective input
            FLAT = STATS_ROWS * 65 // 128
            macc = bigp.tile([128, FLAT], F32)
            mtmp = zb  # zeroing source is dead by now; reuse as merge temp
            nc.sync.dma_start(
                out=macc[:, :],
                in_=stats_bufs[0][:, :].rearrange("a b -> (a b)")
                                       .rearrange("(p k) -> p k", p=128))
            for r in range(1, N_ROT):
                nc.sync.dma_start(
                    out=mtmp[:, :],
                    in_=stats_bufs[r][:, :].rearrange("a b -> (a b)")
                                           .rearrange("(p k) -> p k", p=128))
                nc.vector.tensor_tensor(out=macc[:, :], in0=macc[:, :],
                                        in1=mtmp[:, :], op=alu.add)
            nc.sync.dma_start(
                out=stats[:, :].rearrange("a b -> (a b)")
                               .rearrange("(p k) -> p k", p=128),
                in_=macc[:, :])

            # ---------------- reduce-scatter: each core gets its slice ----
            stats_sl = dram.tile([N_SL, 65], F32)
            nc.gpsimd.collective_compute(
                "ReduceScatter", alu.add,
                replica_groups=[list(range(N_CORES))],
                ins=[stats[:, :]], outs=[stats_sl[:, :]],
            )

            # ---------------- EMA update on the local 512-code slice -------
            es = bigp.tile([128, K_SL, E_DIM], F32)
            nc.sync.dma_start(
                out=es[:, :, :],
                in_=stats_sl[0:N_SL, 0:E_DIM].rearrange(
                    "(k p) d -> p k d", p=128),
            )
            counts = bigp.tile([128, K_SL], F32)
            nc.sync.dma_start(
                out=counts[:, :],
                in_=stats_sl[0:N_SL, E_DIM:E_DIM + 1].rearrange(
                    "(k p) o -> p (k o)", p=128),
            )
            cs_sb = bigp.tile([128, K_SL], F32)
            nc.sync.dma_start(
                out=cs_sb[:, :],
                in_=cs_in[:].rearrange("(k p) -> p k", p=128),
            )
            ea_sb = bigp.tile([128, K_SL, E_DIM], F32)
            nc.sync.dma_start(
                out=ea_sb[:, :, :],
                in_=ea_in[:, :].rearrange("(k p) d -> p k d", p=128),
            )

            # new_cs = fl(cs*0.99) + fl(counts*0.01)
            cs99 = bigp.tile([128, K_SL], F32)
            nc.vector.tensor_scalar_mul(cs99[:, :], cs_sb[:, :], DECAY)
            new_cs = bigp.tile([128, K_SL], F32)
            nc.vector.scalar_tensor_tensor(
                out=new_cs[:, :], in0=counts[:, :], scalar=1.0 - DECAY,
                in1=cs99[:, :], op0=alu.mult, op1=alu.add,
            )
            # new_ea = fl(ea*0.99) + fl(es*0.01)  (overwrite ea_sb)
            nc.vector.tensor_scalar_mul(ea_sb[:, :, :], ea_sb[:, :, :], DECAY)
            new_ea = bigp.tile([128, K_SL, E_DIM], F32)
            nc.vector.scalar_tensor_tensor(
                out=new_ea[:, :, :], in0=es[:, :, :], scalar=1.0 - DECAY,
                in1=ea_sb[:, :, :], op0=alu.mult, op1=alu.add,
            )

            # scalar partials: n, entropy (over the local slice), loss
            ncs_sum = bigp.tile([1, K_SL], F32)
            s_ps = ps_tp.tile([1, K_SL], F32, tag="comb")
            nc.tensor.matmul(out=s_ps[:, :], lhsT=ones_col[:, :],
                             rhs=new_cs[:, :], start=True, stop=True)
            nc.vector.tensor_copy(ncs_sum[:, :], s_ps[:, :])
            n_part = bigp.tile([1, 1], F32)
            nc.vector.tensor_reduce(out=n_part[:, :], in_=ncs_sum[:, :],
                                    axis=mybir.AxisListType.X, op=alu.add)

            avg = bigp.tile([128, K_SL], F32)
            nc.vector.tensor_scalar_mul(avg[:, :], counts[:, :], 1.0 / N_TOK)
            avg_e = bigp.tile([128, K_SL], F32)
            nc.vector.tensor_scalar_add(avg_e[:, :], avg[:, :], 1e-10)
            lg = bigp.tile([128, K_SL], F32)
            nc.scalar.activation(lg[:, :], avg_e[:, :],
                                 mybir.ActivationFunctionType.Ln)
            pl = bigp.tile([128, K_SL], F32)
            nc.vector.tensor_tensor(out=pl[:, :], in0=avg[:, :], in1=lg[:, :],
                                    op=alu.mult)
            pls = bigp.tile([128, 1], F32)
            nc.vector.tensor_reduce(out=pls[:, :], in_=pl[:, :],
                                    axis=mybir.AxisListType.X, op=alu.add)
            e_ps = ps_tp.tile([1, 1], F32, tag="comb")
            nc.tensor.matmul(out=e_ps[:, :], lhsT=ones_col[:, :],
                             rhs=pls[:, :], start=True, stop=True)
            ent_part = bigp.tile([1, 1], F32)
            nc.vector.tensor_copy(ent_part[:, :], e_ps[:, :])

            scal_sb = bigp.tile([1, 8], F32)
            nc.vector.memset(scal_sb[:, :], 0.0)
            nc.vector.tensor_copy(scal_sb[:, 0:1], lpart[:, :])
            nc.vector.tensor_copy(scal_sb[:, 1:2], n_part[:, :])
            nc.vector.tensor_copy(scal_sb[:, 2:3], ent_part[:, :])
            scal_in = dram.tile([1, 8], F32)
            scal_red = dram.tile([1, 8], F32)
            nc.sync.dma_start(out=scal_in[:, :], in_=scal_sb[:, :])
            nc.gpsimd.collective_compute(
                "AllReduce", alu.add,
                replica_groups=[list(range(N_CORES))],
                ins=[scal_in[:, :]], outs=[scal_red[:, :]],
            )
            scal = bigp.tile([1, 8], F32)
            nc.sync.dma_start(out=scal[:, :], in_=scal_red[:, :])

            # n -> broadcast to all partitions via PE
            nb_ps = ps_tp.tile([128, 1], F32, tag="comb")
            nc.tensor.matmul(out=nb_ps[:, :],
                             lhsT=ones_col[0:1, :].to_broadcast([1, 128]),
                             rhs=scal[:, 1:2], start=True, stop=True)
            n_bc = bigp.tile([128, 1], F32)
            nc.vector.tensor_copy(n_bc[:, :], nb_ps[:, :])

            # smoothed = (new_cs + eps) / (n + N_E*eps) * n
            denom = bigp.tile([128, 1], F32)
            nc.vector.tensor_scalar_add(denom[:, :], n_bc[:, :], N_E * EPS)
            rden = bigp.tile([128, 1], F32)
            nc.vector.reciprocal(rden[:, :], denom[:, :])
            smf = bigp.tile([128, K_SL], F32)
            nc.vector.tensor_scalar(
                out=smf[:, :], in0=new_cs[:, :],
                scalar1=EPS, scalar2=rden[:, :1], op0=alu.add, op1=alu.mult,
            )
            sm = bigp.tile([128, K_SL], F32)
            nc.vector.tensor_scalar(
                out=sm[:, :], in0=smf[:, :],
                scalar1=n_bc[:, :1], scalar2=None, op0=alu.mult,
            )
            rsm = bigp.tile([128, K_SL], F32)
            nc.vector.reciprocal(rsm[:, :], sm[:, :])
            new_embed = es  # es fully consumed; reuse the buffer
            nc.vector.tensor_tensor(
                out=new_embed[:, :, :], in0=new_ea[:, :, :],
                in1=rsm[:, :].rearrange("p k -> p k ()").to_broadcast(
                    [128, K_SL, E_DIM]),
                op=alu.mult,
            )

            # perplexity = exp(-entropy_total); loss = BETA * total / N
            ent = bigp.tile([1, 1], F32)
            nc.vector.tensor_scalar_mul(ent[:, :], scal[:, 2:3], -1.0)
            ppl = bigp.tile([1, 1], F32)
            nc.scalar.activation(ppl[:, :], ent[:, :],
                                 mybir.ActivationFunctionType.Exp)
            nc.sync.dma_start(out=ppl_out[:, :], in_=ppl[:, :])
            lossv = bigp.tile([1, 1], F32)
            nc.vector.tensor_scalar(
                out=lossv[:, :], in0=scal[:, 0:1],
                scalar1=1.0 / (N_TOK * E_DIM), scalar2=BETA,
                op0=alu.mult, op1=alu.mult,
            )
            nc.sync.dma_start(out=loss_out[:, :], in_=lossv[:, :])

            # EMA slice outputs
            nc.sync.dma_start(
                out=nemb_out[:, :].rearrange("(k p) d -> p k d", p=128),
                in_=new_embed[:, :, :],
            )
            nc.sync.dma_start(
                out=ncs_out[:].rearrange("(k p) -> p k", p=128),
                in_=new_cs[:, :],
            )
            nc.sync.dma_start(
                out=nea_out[:, :].rearrange("(k p) d -> p k d", p=128),
                in_=new_ea[:, :, :],
            )

            # ---------------- straight-through output ----------------
            outT = bigp.tile([E_DIM, T_LOC], F32, tag="zT_slot")
            for i in range(N_TILES):
                tp = ps_tp.tile([E_DIM, 128], F32, tag="tp")
                nc.tensor.transpose(out=tp[:, :], in_=zf[:, i, 0:E_DIM],
                                    identity=ident[:, :])
                nc.scalar.copy(outT[:, i * 128:(i + 1) * 128], tp[:, :])
            nc.sync.dma_start(out=out_sh[:, :], in_=outT[:, :])

    _split_excess_waits(nc)
    return nc


_NC_CACHE = []


def kernel(z, embed_w, cluster_size, embed_avg):
    z = np.ascontiguousarray(z, dtype=np.float32)
    embed_w = np.ascontiguousarray(embed_w, dtype=np.float32)
    cluster_size = np.ascontiguousarray(cluster_size, dtype=np.float32)
    embed_avg = np.ascontiguousarray(embed_avg, dtype=np.float32)

    if not _NC_CACHE:
        _NC_CACHE.append(build_nc())
    nc = _NC_CACHE[0]

    zr = z.reshape(4, 64, 16384)
    in_maps = []
    for c in range(N_CORES):
        b, half = c // 2, c % 2
        zT_c = np.ascontiguousarray(zr[b, :, half * T_LOC:(half + 1) * T_LOC])
        in_maps.append({
            "zT": zT_c,
            "embed_w": embed_w,
            "cluster_size": np.ascontiguousarray(
                cluster_size[c * 512:(c + 1) * 512]),
            "embed_avg": np.ascontiguousarray(
                embed_avg[c * 512:(c + 1) * 512]),
        })

    res = run_bass_kernel_spmd(nc, in_maps, core_ids=list(range(N_CORES)))
    rs = res.results

    out = np.empty((4, 64, 16384), dtype=np.float32)
    idx = np.empty((N_CORES, T_LOC), dtype=np.int32)
    for c in range(N_CORES):
        b, half = c // 2, c % 2
        out[b, :, half * T_LOC:(half + 1) * T_LOC] = rs[c]["out_sh"]
        idx[c] = rs[c]["idx_t"].T.reshape(T_LOC)
    out = out.reshape(4, 64, 16, 32, 32)
    idx = idx.reshape(N_TOK)

    r0 = rs[0]
    loss = np.float32(r0["loss"][0, 0])
    ppl = np.float32(r0["ppl"][0, 0])
    new_embed = np.concatenate([rs[c]["new_embed"] for c in range(N_CORES)])
    new_cs = np.concatenate([rs[c]["new_cs"] for c in range(N_CORES)])
    new_ea = np.concatenate([rs[c]["new_ea"] for c in range(N_CORES)])
    return (out, loss, ppl, idx, new_embed, new_cs, new_ea)


# revision 15
# speedup vs baseline: 2.6167x; 1.6795x over previous
"""EMA VectorQuantizer forward pass on 8 TRN2 NeuronCores (Bass/Tile).

Data-parallel over tokens: z [4,64,16,32,32] -> 65536 tokens of dim 64,
8192 tokens per core (channel-major shard [64, 8192] is a natural slice
of z's layout). The [4096,64] codebook is replicated. Per core:
  scores = 2*z@e.T via PE fp32 matmuls (tokens on partitions, codes on
  free dim), top-8 via DVE max8/max_index, then an exact fp32 re-ranking
  of the 8 candidates replicating jax's rounding of
  d = (||z||^2 + ||e||^2) - 2*z@e.T (the (zz+ee) double-rounding decides
  ~3% of tokens' grid-ties, so it must be emulated bit-exactly).
  z_q by indirect row gather; counts/embed_sum by per-tile dedup
  (selection-matrix matmul) + serialized indirect scatter-add DMAs; the
  per-core partial stats are AllReduce'd and the EMA buffer update is
  computed redundantly on every core.
"""

import numpy as np

import bass_rust
import concourse.bass as bass
import concourse.mybir as mybir
import concourse.tile as tile_mod
from concourse.bass import IndirectOffsetOnAxis
from concourse.masks import make_identity
from concourse.tile import TileContext
from concourse.tile_rust import add_dep_helper
from concourse.bass_utils import run_bass_kernel_spmd

N_CORES = 8
N_E = 4096
E_DIM = 64
T_LOC = 8192          # tokens per core
N_TILES = T_LOC // 128
BETA = 0.25
DECAY = 0.99
EPS = 1e-05
N_TOK = 65536
STATS_ROWS = 4096
N_SL = N_E // N_CORES     # per-core code slice for the EMA update
K_SL = N_SL // 128        # k-tiles per slice
OFF_ZT = N_E * E_DIM
OFF_EA = OFF_ZT + E_DIM * T_LOC
OFF_CS = OFF_EA = OFF_EA  # placeholder, set below
OFF_EA = N_E * E_DIM + E_DIM * T_LOC
OFF_CS = OFF_EA + N_SL * E_DIM
IN_TOT = OFF_CS + N_SL
P_IDX = E_DIM * T_LOC
P_NEMB = P_IDX + T_LOC
P_NCS = P_NEMB + N_SL * E_DIM
P_NEA = P_NCS + N_SL
P_SCAL = P_NEA + N_SL * E_DIM
OUT_TOT = P_SCAL + 2
F32 = mybir.dt.float32

# ---------------------------------------------------------------------------
# workaround 1: walrus in this container rejects >1 sem wait on the
# TileContext tail drain — pre-absorb the global-clock waits one per drain.

def _patched_drain_and_barrier(self, tick_clock, wait_clock):
    nc = self.nc
    vc = tick_clock.global_clock
    nonzero = [(i, vc[i]) for i in range(len(vc)) if vc[i] > 0]
    for i, t in nonzero:
        pvc = bass_rust.VectorClock([0] * len(vc))
        pvc.require_at_least(i, t)
        nop = nc.sync.drain()
        wait_clock.add_sem_waits(nop.ins, bass_rust.ScopedClock({None: pvc}))
    nc.sync.drain()
    nc.all_engine_barrier()
    assert self.sems is not None
    popped = nc._tile_sem_poison_stack.pop()
    assert popped is self._sem_poison
    nc.clear_and_free_semaphores(list(self.sems.allocated().values()))
    nc.all_engine_barrier()


tile_mod.TileContext._drain_and_barrier = _patched_drain_and_barrier

# ---------------------------------------------------------------------------
# workaround 2: same walrus cap on every other instruction — hoist excess
# semaphore waits onto same-engine NoOps inserted right before it.

_wsplit_ctr = [0]


def _split_excess_waits(nc, max_sem_waits=1):
    for f in nc.m.functions:
        for bb in f.blocks:
            insts = bb.instructions
            new = []
            changed = False
            for inst in insts:
                si = inst.sync_info
                waits = list(si.on_wait) if (si and si.on_wait) else []
                sem_w = [w for w in waits if w.sync_type == "semaphore"]
                other_w = [w for w in waits if w.sync_type != "semaphore"]
                keep = max(0, max_sem_waits - len(other_w))
                if len(sem_w) > keep:
                    excess = sem_w[: len(sem_w) - keep]
                    kept = sem_w[len(sem_w) - keep:]
                    for w in excess:
                        _wsplit_ctr[0] += 1
                        nop = mybir.InstNoOp(
                            name=f"I-wsplit-{_wsplit_ctr[0]}", ins=[], outs=[]
                        )
                        nop.engine = inst.engine
                        nop.sync_info = mybir.SyncInfo(on_wait=[w], on_update=[])
                        new.append(nop)
                    inst.sync_info = mybir.SyncInfo(
                        on_wait=other_w + kept,
                        on_update=list(si.on_update) if si.on_update else [],
                    )
                    changed = True
                new.append(inst)
            if changed:
                insts[:] = new


# ---------------------------------------------------------------------------

def build_nc():
    nc = bass.Bass(trn_type="TRN2", num_devices=N_CORES)
    alu = mybir.AluOpType

    # single packed input/output: the PJRT/axon path here costs ~72ms per
    # array per call, so everything rides in one tensor each way.
    # input layout:  [embed_w (first: indirect-gather src needs offset 0) |
    #                 zT | embed_avg slice | cluster_size slice]
    inp = nc.dram_tensor("inp", [IN_TOT], F32, kind="ExternalInput")
    w_in = inp[0:N_E * E_DIM].rearrange("(r c) -> r c", c=E_DIM)
    zT_in = inp[OFF_ZT:OFF_ZT + E_DIM * T_LOC].rearrange(
        "(c t) -> c t", t=T_LOC)
    ea_in = inp[OFF_EA:OFF_EA + N_SL * E_DIM].rearrange("(r c) -> r c", c=E_DIM)
    cs_in = inp[OFF_CS:OFF_CS + N_SL]

    packed = nc.dram_tensor("packed", [OUT_TOT], F32, kind="ExternalOutput")
    out_sh = packed[0:E_DIM * T_LOC].rearrange("(c t) -> c t", t=T_LOC)
    idxf_out = packed[P_IDX:P_IDX + T_LOC].rearrange("(p k) -> p k", k=N_TILES)
    nemb_out = packed[P_NEMB:P_NEMB + N_SL * E_DIM].rearrange(
        "(k p d) -> p k d", p=128, d=E_DIM)
    ncs_out = packed[P_NCS:P_NCS + N_SL].rearrange("(k p) -> p k", p=128)
    nea_out = packed[P_NEA:P_NEA + N_SL * E_DIM].rearrange(
        "(k p d) -> p k d", p=128, d=E_DIM)
    scal_out = packed[P_SCAL:P_SCAL + 2].rearrange("(a b) -> a b", b=2)

    with TileContext(nc) as tc:
        with (
            tc.tile_pool(name="big", bufs=1) as bigp,      # persistent SBUF
            tc.tile_pool(name="wrk", bufs=2) as wrk,       # per-tile rotating
            tc.tile_pool(name="stg", bufs=2) as stg,       # staged scores
            tc.tile_pool(name="ps_sc", bufs=2, space="PSUM") as ps_sc,
            tc.tile_pool(name="ps_tp", bufs=1, space="PSUM") as ps_tp,
            tc.tile_pool(name="dram", bufs=1, space="DRAM") as dram,
        ):
            # ---------------- constants / setup ----------------
            ident = bigp.tile([128, 128], F32)
            make_identity(nc, ident[:, :])
            iota_row_i = bigp.tile([128, 128], mybir.dt.int32)
            nc.gpsimd.iota(iota_row_i[:, :], pattern=[[1, 128]], base=0,
                           channel_multiplier=0)
            iota_row = bigp.tile([128, 128], F32)
            nc.vector.tensor_copy(iota_row[:, :], iota_row_i[:, :])
            iota_p_i = bigp.tile([128, 1], mybir.dt.int32)
            nc.gpsimd.iota(iota_p_i[:, :], pattern=[[1, 1]], base=0,
                           channel_multiplier=1)
            iota_p = bigp.tile([128, 1], F32)
            nc.vector.tensor_copy(iota_p[:, :], iota_p_i[:, :])
            ltmask = bigp.tile([128, 128], F32)
            nc.vector.tensor_scalar(
                out=ltmask[:, :], in0=iota_row[:, :],
                scalar1=iota_p[:, :1], scalar2=None, op0=alu.is_gt,
            )
            ones_col = bigp.tile([128, 1], F32)
            nc.vector.memset(ones_col[:, :], 1.0)
            ident64 = bigp.tile([64, 64], F32)
            make_identity(nc, ident64[:, :])

            # ---------------- load z (channel-major) and codebook ----------
            zT = bigp.tile([E_DIM, T_LOC], F32, tag="zT_slot")
            nc.sync.dma_start(out=zT[:, :], in_=zT_in)
            e_sb = bigp.tile([128, 32, E_DIM], F32)
            nc.sync.dma_start(
                out=e_sb[:, :, :],
                in_=w_in.rearrange("(k p) d -> p k d", p=128),
            )

            # e2T[64, 4096] = 2 * e.T ; ee[4096] = rowwise ||e||^2 (DRAM)
            e2T = bigp.tile([E_DIM, N_E], F32)
            ee_cols = bigp.tile([128, 32], F32)
            scr = bigp.tile([128, E_DIM], F32)
            ee_dram = dram.tile([1, N_E], F32)
            for k in range(32):
                tp = ps_tp.tile([E_DIM, 128], F32, tag="tp")
                nc.tensor.transpose(out=tp[:, :], in_=e_sb[:, k, :],
                                    identity=ident[:, :])
                nc.scalar.mul(e2T[:, k * 128:(k + 1) * 128], tp[:, :], 2.0)
                nc.scalar.activation(
                    scr[:, :], e_sb[:, k, :],
                    mybir.ActivationFunctionType.Square,
                    accum_out=ee_cols[:, k:k + 1],
                )
            nc.sync.dma_start(
                out=ee_dram[0:1, :].rearrange("o (k p) -> (o p) k", p=128),
                in_=ee_cols[:, :],
            )
            negEE = bigp.tile([128, N_E], F32)
            nc.sync.dma_start(
                out=negEE[:, :],
                in_=ee_dram[0:1, :].to_broadcast([128, N_E]),
            )
            nc.vector.tensor_scalar_mul(negEE[:, :], negEE[:, :], -1.0)

            # zf (token-major) + zz per token
            zf = bigp.tile([128, N_TILES, E_DIM + 1], F32)
            nc.vector.memset(zf[:, :, E_DIM:E_DIM + 1], 1.0)
            zz_all = bigp.tile([128, N_TILES], F32)
            for i in range(N_TILES):
                tp = ps_tp.tile([128, E_DIM], F32, tag="tp")
                nc.tensor.transpose(
                    out=tp[:, :], in_=zT[:, i * 128:(i + 1) * 128],
                    identity=ident64[:, :],
                )
                nc.vector.tensor_copy(zf[:, i, 0:E_DIM], tp[:, :])
                nc.scalar.activation(
                    scr[:, :], zf[:, i, 0:E_DIM],
                    mybir.ActivationFunctionType.Square,
                    accum_out=zz_all[:, i:i + 1],
                )

            # stats accumulator in DRAM (zeroed): rows 0..4095 = [embed_sum |
            # count]; row 4096 col 0 = loss partial
            negzz_all = bigp.tile([128, N_TILES], F32)
            nc.vector.tensor_scalar_mul(negzz_all[:, :], zz_all[:, :], -1.0)
            N_ROT = 8
            stats = dram.tile([STATS_ROWS, 65], F32)
            stats_bufs = []
            for r in range(N_ROT):
                srot = dram.tile([STATS_ROWS, 65], F32, tag=f"srot{r}")
                stats_bufs.append(srot)
            zb = bigp.tile([128, STATS_ROWS * 65 // 128], F32)
            nc.vector.memset(zb[:, :], 0.0)
            znop = nc.gpsimd.engine_nop()
            for sb in stats_bufs:
                zd = nc.sync.dma_start(
                    out=sb[:, :].rearrange("a b -> (a b)")
                                .rearrange("(p k) -> p k", p=128),
                    in_=zb[:, :],
                )
                add_dep_helper(znop.ins, zd.ins, sync=True,
                               reason="gpsimd observes stats zeroing")

            bc_reg = nc.gpsimd.to_reg(N_E - 1)
            idx_all = bigp.tile([128, N_TILES], mybir.dt.int32)
            idx_allf = bigp.tile([128, N_TILES], F32)
            lcols = bigp.tile([128, N_TILES], F32)
            comb_all = bigp.tile([128, N_TILES, 65], F32)
            ieff_all = bigp.tile([128, N_TILES], mybir.dt.int32)

            # ---------------- main loop over 64 token tiles ----------------
            for i in range(N_TILES):
                lhsT = zT[:, i * 128:(i + 1) * 128]
                staged = stg.tile([128, N_E + 8], F32, tag="staged")
                nc.vector.memset(staged[:, N_E:N_E + 8], -1e30)
                for base, width in ((0, 1536), (1536, 1536), (3072, 1024)):
                    ps = ps_sc.tile([128, width], F32, tag="sc")
                    for c in range(width // 512):
                        col = base + c * 512
                        nc.tensor.matmul(
                            out=ps[:, c * 512:(c + 1) * 512],
                            lhsT=lhsT,
                            rhs=e2T[:, col:col + 512],
                            start=True, stop=True,
                        )
                    # staged = fl(fl(-ee - zz) + 2*z@e) = -d with jax's
                    # exact double rounding; max over it = argmin of d
                    nc.vector.scalar_tensor_tensor(
                        out=staged[:, base:base + width],
                        in0=negEE[:, base:base + width],
                        scalar=negzz_all[:, i:i + 1],
                        in1=ps[:, 0:width],
                        op0=alu.add, op1=alu.add,
                    )

                v8 = wrk.tile([128, 8], F32, tag="v8")
                nc.vector.max(out=v8[:, :], in_=staged[:, 0:N_E + 8])
                i8 = wrk.tile([128, 8], mybir.dt.uint32, tag="i8")
                nc.vector.max_index(out=i8[:, :], in_max=v8[:, :],
                                    in_values=staged[:, 0:N_E + 8])
                i8f = wrk.tile([128, 8], F32, tag="i8f")
                nc.vector.tensor_copy(i8f[:, 0:1], i8[:, 0:1])
                nc.vector.tensor_copy(idx_allf[:, i:i + 1], i8f[:, 0:1])
                nc.vector.tensor_copy(idx_all[:, i:i + 1], i8f[:, 0:1])

                # z_q gather, straight-through out, loss contribution
                zq = wrk.tile([128, E_DIM], F32, tag="zq")
                nc.gpsimd.indirect_dma_start(
                    out=zq[:, :], out_offset=None,
                    in_=w_in,
                    in_offset=IndirectOffsetOnAxis(ap=idx_all[:, i:i + 1], axis=0),
                )
                dlt = wrk.tile([128, E_DIM], F32, tag="dlt")
                nc.vector.tensor_tensor(out=dlt[:, :], in0=zq[:, :],
                                        in1=zf[:, i, 0:E_DIM], op=alu.subtract)
                nc.scalar.activation(
                    scr[:, :], dlt[:, :],
                    mybir.ActivationFunctionType.Square,
                    accum_out=lcols[:, i:i + 1],
                )

                # in-tile dedup for the scatter: S = (idx == idx.T)
                tp = ps_tp.tile([128, 128], F32, tag="tp")
                nc.tensor.transpose(
                    out=tp[:, :],
                    in_=idx_allf[:, i:i + 1].to_broadcast([128, 128]),
                    identity=ident[:, :],
                )
                idxT = wrk.tile([128, 128], F32, tag="idxT")
                nc.vector.tensor_copy(idxT[:, :], tp[:, :])
                S = wrk.tile([128, 128], F32, tag="S")
                nc.vector.tensor_tensor(
                    out=S[:, :],
                    in0=idx_allf[:, i:i + 1].to_broadcast([128, 128]),
                    in1=idxT[:, :], op=alu.is_equal,
                )
                comb_ps = ps_tp.tile([128, 65], F32, tag="comb")
                nc.tensor.matmul(out=comb_ps[:, :], lhsT=S[:, :],
                                 rhs=zf[:, i, :], start=True, stop=True)
                nc.vector.tensor_copy(comb_all[:, i, :], comb_ps[:, :])
                SL = wrk.tile([128, 128], F32, tag="SL")
                nc.vector.tensor_tensor(out=SL[:, :], in0=S[:, :],
                                        in1=ltmask[:, :], op=alu.mult)
                cnt = wrk.tile([128, 1], F32, tag="cnt")
                nc.vector.tensor_reduce(out=cnt[:, :], in_=SL[:, :],
                                        axis=mybir.AxisListType.X, op=alu.add)
                notfirst = wrk.tile([128, 1], F32, tag="nf")
                nc.vector.tensor_scalar(
                    out=notfirst[:, :], in0=cnt[:, :],
                    scalar1=0.0, scalar2=None, op0=alu.is_gt,
                )
                idx_eff_f = wrk.tile([128, 1], F32, tag="ieff_f")
                nc.vector.scalar_tensor_tensor(
                    out=idx_eff_f[:, :], in0=notfirst[:, :], scalar=60000.0,
                    in1=idx_allf[:, i:i + 1], op0=alu.mult, op1=alu.add,
                )
                nc.vector.tensor_copy(ieff_all[:, i:i + 1], idx_eff_f[:, :])
                sc_dma = nc.gpsimd.indirect_dma_start(
                    out=stats_bufs[i % N_ROT][:, :],
                    out_offset=IndirectOffsetOnAxis(ap=ieff_all[:, i:i + 1], axis=0),
                    in_=comb_all[:, i, :], in_offset=None,
                    compute_op=alu.add,
                    bounds_check=bc_reg, oob_is_err=False,
                )
                add_dep_helper(sc_dma.ins, znop.ins, sync=False,
                               reason="scatter after stats zeroing")

                # straight-through output overwrites zf slot:
                # out = zp + (z_q - zp), computed exactly like jax
                nc.vector.tensor_tensor(out=zf[:, i, 0:E_DIM],
                                        in0=zf[:, i, 0:E_DIM],
                                        in1=dlt[:, :], op=alu.add)

            # idx output (as fp32; host casts back, values <= 4095 exact)
            nc.sync.dma_start(out=idxf_out, in_=idx_allf[:, :])

            # ---------------- loss partial ----------------
            lsum = bigp.tile([128, 1], F32)
            nc.vector.tensor_reduce(out=lsum[:, :], in_=lcols[:, :],
                                    axis=mybir.AxisListType.X, op=alu.add)
            lt_ps = ps_tp.tile([1, 1], F32, tag="comb")
            nc.tensor.matmul(out=lt_ps[:, :], lhsT=ones_col[:, :],
                             rhs=lsum[:, :], start=True, stop=True)
            lpart = bigp.tile([1, 1], F32)
            nc.vector.tensor_copy(lpart[:, :], lt_ps[:, :])

            # merge the rotated partials into the collective input
            FLAT = STATS_ROWS * 65 // 128
            macc = bigp.tile([128, FLAT], F32)
            mtmp = zb  # zeroing source is dead by now; reuse as merge temp
            nc.sync.dma_start(
                out=macc[:, :],
                in_=stats_bufs[0][:, :].rearrange("a b -> (a b)")
                                       .rearrange("(p k) -> p k", p=128))
            for r in range(1, N_ROT):
                nc.sync.dma_start(
                    out=mtmp[:, :],
                    in_=stats_bufs[r][:, :].rearrange("a b -> (a b)")
                                           .rearrange("(p k) -> p k", p=128))
                nc.vector.tensor_tensor(out=macc[:, :], in0=macc[:, :],
                                        in1=mtmp[:, :], op=alu.add)
            nc.sync.dma_start(
                out=stats[:, :].rearrange("a b -> (a b)")
                               .rearrange("(p k) -> p k", p=128),
                in_=macc[:, :])

            # ---------------- reduce-scatter: each core gets its slice ----
            stats_sl = dram.tile([N_SL, 65], F32)
            nc.gpsimd.collective_compute(
                "ReduceScatter", alu.add,
                replica_groups=[list(range(N_CORES))],
                ins=[stats[:, :]], outs=[stats_sl[:, :]],
            )

            # ---------------- EMA update on the local 512-code slice -------
            es = bigp.tile([128, K_SL, E_DIM], F32)
            nc.sync.dma_start(
                out=es[:, :, :],
                in_=stats_sl[0:N_SL, 0:E_DIM].rearrange(
                    "(k p) d -> p k d", p=128),
            )
            counts = bigp.tile([128, K_SL], F32)
            nc.sync.dma_start(
                out=counts[:, :],
                in_=stats_sl[0:N_SL, E_DIM:E_DIM + 1].rearrange(
                    "(k p) o -> p (k o)", p=128),
            )
            cs_sb = bigp.tile([128, K_SL], F32)
            nc.sync.dma_start(
                out=cs_sb[:, :],
                in_=cs_in.rearrange("(k p) -> p k", p=128),
            )
            ea_sb = bigp.tile([128, K_SL, E_DIM], F32)
            nc.sync.dma_start(
                out=ea_sb[:, :, :],
                in_=ea_in.rearrange("(k p) d -> p k d", p=128),
            )

            # new_cs = fl(cs*0.99) + fl(counts*0.01)
            cs99 = bigp.tile([128, K_SL], F32)
            nc.vector.tensor_scalar_mul(cs99[:, :], cs_sb[:, :], DECAY)
            new_cs = bigp.tile([128, K_SL], F32)
            nc.vector.scalar_tensor_tensor(
                out=new_cs[:, :], in0=counts[:, :], scalar=1.0 - DECAY,
                in1=cs99[:, :], op0=alu.mult, op1=alu.add,
            )
            # new_ea = fl(ea*0.99) + fl(es*0.01)  (overwrite ea_sb)
            nc.vector.tensor_scalar_mul(ea_sb[:, :, :], ea_sb[:, :, :], DECAY)
            new_ea = bigp.tile([128, K_SL, E_DIM], F32)
            nc.vector.scalar_tensor_tensor(
                out=new_ea[:, :, :], in0=es[:, :, :], scalar=1.0 - DECAY,
                in1=ea_sb[:, :, :], op0=alu.mult, op1=alu.add,
            )

            # scalar partials: n, entropy (over the local slice), loss
            ncs_sum = bigp.tile([1, K_SL], F32)
            s_ps = ps_tp.tile([1, K_SL], F32, tag="comb")
            nc.tensor.matmul(out=s_ps[:, :], lhsT=ones_col[:, :],
                             rhs=new_cs[:, :], start=True, stop=True)
            nc.vector.tensor_copy(ncs_sum[:, :], s_ps[:, :])
            n_part = bigp.tile([1, 1], F32)
            nc.vector.tensor_reduce(out=n_part[:, :], in_=ncs_sum[:, :],
                                    axis=mybir.AxisListType.X, op=alu.add)

            avg = bigp.tile([128, K_SL], F32)
            nc.vector.tensor_scalar_mul(avg[:, :], counts[:, :], 1.0 / N_TOK)
            avg_e = bigp.tile([128, K_SL], F32)
            nc.vector.tensor_scalar_add(avg_e[:, :], avg[:, :], 1e-10)
            lg = bigp.tile([128, K_SL], F32)
            nc.scalar.activation(lg[:, :], avg_e[:, :],
                                 mybir.ActivationFunctionType.Ln)
            pl = bigp.tile([128, K_SL], F32)
            nc.vector.tensor_tensor(out=pl[:, :], in0=avg[:, :], in1=lg[:, :],
                                    op=alu.mult)
            pls = bigp.tile([128, 1], F32)
            nc.vector.tensor_reduce(out=pls[:, :], in_=pl[:, :],
                                    axis=mybir.AxisListType.X, op=alu.add)
            e_ps = ps_tp.tile([1, 1], F32, tag="comb")
            nc.tensor.matmul(out=e_ps[:, :], lhsT=ones_col[:, :],
                             rhs=pls[:, :], start=True, stop=True)
            ent_part = bigp.tile([1, 1], F32)
            nc.vector.tensor_copy(ent_part[:, :], e_ps[:, :])

            scal_sb = bigp.tile([1, 8], F32)
            nc.vector.memset(scal_sb[:, :], 0.0)
            nc.vector.tensor_copy(scal_sb[:, 0:1], lpart[:, :])
            nc.vector.tensor_copy(scal_sb[:, 1:2], n_part[:, :])
            nc.vector.tensor_copy(scal_sb[:, 2:3], ent_part[:, :])
            scal_in = dram.tile([1, 8], F32)
            scal_red = dram.tile([1, 8], F32)
            nc.sync.dma_start(out=scal_in[:, :], in_=scal_sb[:, :])
            nc.gpsimd.collective_compute(
                "AllReduce", alu.add,
                replica_groups=[list(range(N_CORES))],
                ins=[scal_in[:, :]], outs=[scal_red[:, :]],
            )
            scal = bigp.tile([1, 8], F32)
            nc.sync.dma_start(out=scal[:, :], in_=scal_red[:, :])

            # n -> broadcast to all partitions via PE
            nb_ps = ps_tp.tile([128, 1], F32, tag="comb")
            nc.tensor.matmul(out=nb_ps[:, :],
                             lhsT=ones_col[0:1, :].to_broadcast([1, 128]),
                             rhs=scal[:, 1:2], start=True, stop=True)
            n_bc = bigp.tile([128, 1], F32)
            nc.vector.tensor_copy(n_bc[:, :], nb_ps[:, :])

            # smoothed = (new_cs + eps) / (n + N_E*eps) * n
            denom = bigp.tile([128, 1], F32)
            nc.vector.tensor_scalar_add(denom[:, :], n_bc[:, :], N_E * EPS)
            rden = bigp.tile([128, 1], F32)
            nc.vector.reciprocal(rden[:, :], denom[:, :])
            smf = bigp.tile([128, K_SL], F32)
            nc.vector.tensor_scalar(
                out=smf[:, :], in0=new_cs[:, :],
                scalar1=EPS, scalar2=rden[:, :1], op0=alu.add, op1=alu.mult,
            )
            sm = bigp.tile([128, K_SL], F32)
            nc.vector.tensor_scalar(
                out=sm[:, :], in0=smf[:, :],
                scalar1=n_bc[:, :1], scalar2=None, op0=alu.mult,
            )
            rsm = bigp.tile([128, K_SL], F32)
            nc.vector.reciprocal(rsm[:, :], sm[:, :])
            new_embed = es  # es fully consumed; reuse the buffer
            nc.vector.tensor_tensor(
                out=new_embed[:, :, :], in0=new_ea[:, :, :],
                in1=rsm[:, :].rearrange("p k -> p k ()").to_broadcast(
                    [128, K_SL, E_DIM]),
                op=alu.mult,
            )

            # perplexity = exp(-entropy_total); loss = BETA * total / N
            ent = bigp.tile([1, 1], F32)
            nc.vector.tensor_scalar_mul(ent[:, :], scal[:, 2:3], -1.0)
            lp2 = bigp.tile([1, 2], F32)
            nc.vector.tensor_scalar(
                out=lp2[:, 0:1], in0=scal[:, 0:1],
                scalar1=1.0 / (N_TOK * E_DIM), scalar2=BETA,
                op0=alu.mult, op1=alu.mult,
            )
            nc.scalar.activation(lp2[:, 1:2], ent[:, :],
                                 mybir.ActivationFunctionType.Exp)
            nc.sync.dma_start(out=scal_out, in_=lp2[:, :])

            # EMA slice outputs
            nc.sync.dma_start(out=nemb_out, in_=new_embed[:, :, :])
            nc.sync.dma_start(out=ncs_out, in_=new_cs[:, :])
            nc.sync.dma_start(out=nea_out, in_=new_ea[:, :, :])

            # ---------------- straight-through output ----------------
            outT = bigp.tile([E_DIM, T_LOC], F32, tag="zT_slot")
            for i in range(N_TILES):
                tp = ps_tp.tile([E_DIM, 128], F32, tag="tp")
                nc.tensor.transpose(out=tp[:, :], in_=zf[:, i, 0:E_DIM],
                                    identity=ident[:, :])
                nc.scalar.copy(outT[:, i * 128:(i + 1) * 128], tp[:, :])
            nc.sync.dma_start(out=out_sh, in_=outT[:, :])

    _split_excess_waits(nc)
    return nc


_NC_CACHE = []


def kernel(z, embed_w, cluster_size, embed_avg):
    z = np.ascontiguousarray(z, dtype=np.float32)
    embed_w = np.ascontiguousarray(embed_w, dtype=np.float32)
    cluster_size = np.ascontiguousarray(cluster_size, dtype=np.float32)
    embed_avg = np.ascontiguousarray(embed_avg, dtype=np.float32)

    if not _NC_CACHE:
        _NC_CACHE.append(build_nc())
    nc = _NC_CACHE[0]

    zr = z.reshape(4, 64, 16384)
    in_maps = []
    for c in range(N_CORES):
        b, half = c // 2, c % 2
        zT_c = np.ascontiguousarray(zr[b, :, half * T_LOC:(half + 1) * T_LOC])
        in_maps.append({"inp": np.concatenate([
            embed_w.ravel(), zT_c.ravel(),
            embed_avg[c * 512:(c + 1) * 512].ravel(),
            cluster_size[c * 512:(c + 1) * 512],
        ])})

    res = run_bass_kernel_spmd(nc, in_maps, core_ids=list(range(N_CORES)))
    rs = res.results

    out = np.empty((4, 64, 16384), dtype=np.float32)
    idx = np.empty((N_CORES, T_LOC), dtype=np.int32)
    nemb, ncs, nea = [], [], []
    for c in range(N_CORES):
        b, half = c // 2, c % 2
        pk = rs[c]["packed"]
        out[b, :, half * T_LOC:(half + 1) * T_LOC] = \
            pk[0:E_DIM * T_LOC].reshape(E_DIM, T_LOC)
        idx[c] = pk[P_IDX:P_IDX + T_LOC].reshape(128, N_TILES) \
            .T.reshape(T_LOC).astype(np.int32)
        nemb.append(pk[P_NEMB:P_NEMB + N_SL * E_DIM].reshape(N_SL, E_DIM))
        ncs.append(pk[P_NCS:P_NCS + N_SL])
        nea.append(pk[P_NEA:P_NEA + N_SL * E_DIM].reshape(N_SL, E_DIM))
    out = out.reshape(4, 64, 16, 32, 32)
    idx = idx.reshape(N_TOK)
    loss = np.float32(rs[0]["packed"][P_SCAL])
    ppl = np.float32(rs[0]["packed"][P_SCAL + 1])
    return (out, loss, ppl, idx, np.concatenate(nemb),
            np.concatenate(ncs), np.concatenate(nea))
